# revision 15
# baseline (speedup 1.0000x reference)
"""Causal single-head attention (B=4, T=4096, E=1024, D=64) on 8 trn2 NeuronCores.

Strategy (bf16 rework of the fp32r baseline; ~1.5-1.6x faster):
  - 2 cores per batch, causally-balanced q split: "outer" core owns q-chunks
    {0,1,6,7} (needs kv chunks 0..7), "middle" owns {2,3,4,5} (kv 0..5).
    Both do 72 score/AV k-blocks.
  - All matmul operands bf16 (x packed to bf16 on host): halves DMA and SBUF
    traffic; PSUM accumulation stays fp32.
  - Natural chunk stream 0,1,2,...: when chunk G (a q-chunk) arrives, all kv
    chunks <= G are already on-chip, so q-chunk G's attention runs immediately
    and completely -> at most one live AV accumulator (+1 draining).
  - Scores computed transposed (S^T[k, q]) with k-chunks folded by parity onto
    partition halves (even chunks' kT on partitions 0:64, odd on 64:128, via
    host-stacked [Wk|Wv] / [Wv|Wk] weights), so score matmuls (K=64) run
    pairwise-concurrent via PE row tiling.
  - Score pairs land in a 2-bank fp32 PSUM tile [128, 1024]; ONE ACT exp
    instruction converts the whole group to bf16 e_t (amortizes the ~300ns
    ACT instruction overhead).
  - Softmax denominator = 65th "ones" column of v in the AV matmul; the
    kernel ships the unnormalized outT [65, 512] per q-chunk (row 64 = the
    denominator) and the host does the divide + layout transpose.
  - Diagonal-block causal masks multiply on the otherwise-idle GPSIMD engine
    so the DVE queue never serializes behind the ACT exp chain.

Two programs (outer/middle) run 4-core SPMD on disjoint device sets.
"""

import functools

import numpy as np

import concourse.bass as bass
import concourse.mybir as mybir
import concourse.tile as tile
from concourse import bacc
from concourse.masks import make_identity

E = 1024
D = 64
T = 4096
B = 4
CH = 512  # 512-row x/q/kv chunk
NB = 128  # k block size (PE partition dim of score output)
SCALE = 1.0 / 32.0  # E ** -0.5

OUTER_GIDS = (0, 1, 6, 7)
MIDDLE_GIDS = (2, 3, 4, 5)
OUTER_NKV = 8
MIDDLE_NKV = 6

FP32 = mybir.dt.float32
FP32R = mybir.dt.float32r
BF16 = mybir.dt.bfloat16


def _build_body(ctx, tc, xT, wa, wb, wq2, out, n_kv, q_gids, repeat=1,
                hw_loop=False, stage="full", dma_split=True, sp_banks=2,
                sp_bufs=None, dma_group=1, prj_bufs=2, vt_share=True,
                out_combined=False, pump_n=2, av_lag=2, dma_rings=2,
                acc_bufs=2, vt_pool="acc", unroll=8, xc_bufs=None):
    nc = tc.nc
    q_local = {g: i for i, g in enumerate(q_gids)}
    if sp_bufs is None:
        sp_bufs = 4 // sp_banks

    pers = ctx.enter_context(tc.tile_pool(name="pers", bufs=1))
    n_xgrp = -(-n_kv // dma_group)
    xc_pool = ctx.enter_context(
        tc.tile_pool(name="xc", bufs=xc_bufs or max(2, n_xgrp))
    )
    exp_pool = ctx.enter_context(tc.tile_pool(name="expp", bufs=sp_bufs + 1))
    sm_pool = ctx.enter_context(tc.tile_pool(name="sm", bufs=2))
    ps_sp = ctx.enter_context(tc.tile_pool(name="ps_sp", bufs=sp_bufs, space="PSUM"))
    ps_acc = ctx.enter_context(tc.tile_pool(name="ps_acc", bufs=acc_bufs, space="PSUM"))
    ps_pr = ctx.enter_context(tc.tile_pool(name="ps_pr", bufs=prj_bufs, space="PSUM"))
    if vt_pool == "acc" and vt_share:
        ps_vt, vt_tag = ps_acc, "acc"
    elif vt_pool == "prj":
        ps_vt, vt_tag = ps_pr, "prj"
    else:
        ps_vt = ctx.enter_context(tc.tile_pool(name="ps_vt", bufs=1, space="PSUM"))
        vt_tag = "vt4"

    # ---- persistent SBUF tensors ----
    wa_sb = pers.tile([128, E], BF16, tag="wa")   # [Wk|Wv] per e-block (even chunks)
    wb_sb = pers.tile([128, E], BF16, tag="wb")   # [Wv|Wk] per e-block (odd chunks)
    wq_sb = pers.tile([128, E], BF16, tag="wq2")  # [Wq|Wq] per e-block
    kv_sb = pers.tile([128, n_kv * CH], BF16, tag="kvsb")  # folded kT/vT per chunk
    qtd = pers.tile([128, len(q_gids) * CH], BF16, tag="qtd")  # dup'd qT per q-chunk
    v_sb = pers.tile([128, 4 * n_kv * (D + 1)], BF16, tag="vsb")  # [v | 1] blocks
    masks = pers.tile([128, 4 * CH], BF16, tag="masks")
    ident = pers.tile([128, 128], BF16, tag="ident")
    ones65 = pers.tile([1, D + 1], FP32R, tag="ones65")

    make_identity(nc, ident[:])

    # staircase causal masks M_j[r, c] = 1 iff c - r - 128*j >= 0
    nc.gpsimd.memset(masks[:], 1.0)
    for j in range(4):
        nc.gpsimd.affine_select(
            out=masks[:, CH * j : CH * (j + 1)],
            in_=masks[:, CH * j : CH * (j + 1)],
            compare_op=mybir.AluOpType.is_ge,
            fill=0.0,
            base=-NB * j,
            channel_multiplier=-1,
            pattern=[[1, CH]],
        )

    # v ones-columns + ones row for the reciprocal broadcast matmul
    n_blocks = 4 * n_kv
    ones_view = v_sb[:].rearrange("p (b c) -> p b c", c=D + 1)[:, :, D : D + 1]
    const1 = nc.const_aps.tensor(1.0, (128, n_blocks, 1), FP32)
    nc.scalar.activation(
        ones_view, const1, mybir.ActivationFunctionType.Copy, bias=0.0, scale=1.0
    )
    const1r = nc.const_aps.tensor(1.0, (1, D + 1), FP32)
    nc.scalar.activation(
        ones65[:], const1r, mybir.ActivationFunctionType.Copy, bias=0.0, scale=1.0
    )

    # weights arrive pre-stacked from host: [128, 8*128] bf16
    nc.sync.dma_start(wa_sb[:], wa)
    nc.sync.dma_start(wb_sb[:], wb)
    nc.sync.dma_start(wq_sb[:], wq2)

    if stage not in ("full", "dma", "proj", "noexp", "empty"):  # microbenches
        nc.gpsimd.memset(kv_sb[:], 0.001)
        nc.gpsimd.memset(qtd[:], 0.001)
        nc.gpsimd.memset(v_sb[:], 0.001)

    def epilogue(G, acc, osb_all):
        # stash the unnormalized outT [65, 512] (row 64 = softmax denominator);
        # the host divides + transposes.
        qi = q_local[G]
        if out_combined:
            # one combined out-DMA ships all four at iteration end
            nc.vector.tensor_copy(osb_all[:, CH * qi : CH * (qi + 1)], acc[:])
        else:
            osb = sm_pool.tile([D + 1, CH], FP32, tag="osb1")
            nc.vector.tensor_copy(osb[:], acc[:])
            nc.scalar.dma_start(out[(D + 1) * qi : (D + 1) * (qi + 1), :], osb[:])

    def attention_gen(G, osb_all, lag=None):
        """Full attention for q-chunk G (all kv chunks <= G are on-chip).
        Generator: yields after each score-group emission so the caller can
        interleave projection work into the PE stream (the in-order PE FIFO
        otherwise stalls on the scores->exp->AV chain)."""
        from collections import deque

        qi = q_local[G]
        qcols = slice(CH * qi, CH * (qi + 1))
        evens = [c for c in range(G + 1) if c % 2 == 0]
        odds = [c for c in range(G + 1) if c % 2 == 1]
        # groups: [(ce, co)] pairs then leftover singles, x4 j-blocks each;
        # groups touching the diagonal chunk G go FIRST so the GPSIMD mask
        # latency hides in pipeline fill instead of gating the last AVs
        groups = []
        for ce, co in zip(evens, odds):
            for j in range(4):
                groups.append(((ce, j), (co, j)))
        for c in evens[len(odds):] + odds[len(evens):]:
            for j in range(4):
                groups.append(((c, j),))

        if lag is None:
            lag = av_lag
        acc = ps_acc.tile([D + 1, CH], FP32, tag="acc", name="acc")
        n_units = 4 * (G + 1)
        done = 0
        prevq = deque()

        def flush_one():
            nonlocal done
            ets, units, qofs = prevq.popleft()
            for et, (c, j), o in zip(ets, units, qofs):
                blk = 4 * c + j
                # partial-width AV writes acc cols [o:CH]; safe because the
                # first AV of the chunk (start=True) is always full-width
                nc.tensor.matmul(
                    acc[:, o:CH],
                    v_sb[:, (D + 1) * blk : (D + 1) * (blk + 1)],
                    et,
                    start=(done == 0),
                    stop=(done == n_units - 1),
                )
                done += 1

        for units in groups:
            # diagonal blocks (c == G): q-cols [0, 128j) are entirely below
            # the causal mask -- skip them (scores, exp, AV all narrower).
            qofs = [NB * j if c == G else 0 for c, j in units]
            ws = [CH - o for o in qofs]
            sofs = [sum(ws[:s]) for s in range(len(units))]
            tot = sum(ws)
            sp = ps_sp.tile([128, 2 * CH], FP32, tag="sp", name="sp")
            e_t = exp_pool.tile([128, 2 * CH], BF16, tag="et", name="e_t")
            sps = [sp[:, sofs[s] : sofs[s] + ws[s]] for s in range(len(units))]
            ets = [e_t[:, sofs[s] : sofs[s] + ws[s]] for s in range(len(units))]
            for s, (c, j) in enumerate(units):
                half = 64 * (c % 2)
                nc.tensor.matmul(
                    sps[s],
                    kv_sb[half : half + 64, CH * c + NB * j : CH * c + NB * (j + 1)],
                    qtd[half : half + 64, CH * qi + qofs[s] : CH * (qi + 1)],
                    start=True,
                    stop=True,
                )
            if stage == "noexp":
                nc.vector.tensor_copy(e_t[:, 0:tot], sp[:, 0:tot])
            else:
                nc.scalar.activation(
                    e_t[:, 0:tot], sp[:, 0:tot],
                    mybir.ActivationFunctionType.Exp, bias=0.0, scale=SCALE,
                )
            for s, (c, j) in enumerate(units):
                if c == G:  # partial causal mask (on idle GPSIMD so the DVE
                    # queue never stalls behind the exp chain)
                    nc.gpsimd.tensor_mul(
                        ets[s], ets[s],
                        masks[:, CH * j + qofs[s] : CH * (j + 1)],
                    )
            prevq.append((ets, units, qofs))
            if len(prevq) > lag:
                flush_one()
            yield
        while prevq:
            flush_one()
        epilogue(G, acc, osb_all)
        yield

    def one_rep_micro():
        if stage == "empty":
            t = sm_pool.tile([128, 1], FP32, tag="osb")
            nc.vector.tensor_copy(t[:], masks[:, 0:1])
            return
        if stage == "mm":  # proj-like PE stream, no DMA/copy deps
            for c in range(n_kv):
                kv_ps = ps_pr.tile([128, CH], FP32, tag="prj", name="kv_ps")
                for eb in range(8):
                    nc.tensor.matmul(
                        kv_ps[:], wa_sb[:, 128 * eb : 128 * (eb + 1)],
                        kv_sb[:, CH * eb : CH * (eb + 1)] if n_kv >= 8 else kv_sb[:, 0:CH],
                        start=(eb == 0), stop=(eb == 7),
                    )
            return
        if stage == "mmt":  # proj MMs + transposes
            for c in range(n_kv):
                kv_ps = ps_pr.tile([128, CH], FP32, tag="prj", name="kv_ps")
                for eb in range(8):
                    nc.tensor.matmul(
                        kv_ps[:], wa_sb[:, 128 * eb : 128 * (eb + 1)],
                        kv_sb[:, CH * eb : CH * (eb + 1)] if n_kv >= 8 else kv_sb[:, 0:CH],
                        start=(eb == 0), stop=(eb == 7),
                    )
                vt4 = ps_vt.tile([128, 4 * D], BF16, tag=vt_tag, name="vt4")
                for j in range(4):
                    nc.tensor.transpose(
                        vt4[:, D * j : D * (j + 1)],
                        kv_sb[0:64, CH * c + NB * j : CH * c + NB * (j + 1)],
                        ident[0:64, 0:64],
                    )
            return
        if stage == "exp":  # ACT-only: 36 exps of [128, 1024] from SBUF
            for g in range(36):
                e_t = exp_pool.tile([128, 2 * CH], BF16, tag="et", name="e_t")
                nc.scalar.activation(
                    e_t[:], kv_sb[:, 0 : 2 * CH],
                    mybir.ActivationFunctionType.Exp, bias=0.0, scale=SCALE,
                )
            return
        if stage == "att":  # attention only (kv_sb/qtd hold garbage)
            osb_all = sm_pool.tile([D + 1, 4 * CH], FP32, tag="osb", name="osb_all")
            for g in list(q_local):
                for _ in attention_gen(g, osb_all):
                    pass
            nc.scalar.dma_start(
                out[:].rearrange("(q p) m -> p q m", p=D + 1),
                osb_all[:].rearrange("p (q m) -> p q m", q=len(q_gids)),
            )
            return
        if stage == "attmmt":  # attention + dependency-free proj-like PE work
            osb_all = sm_pool.tile([D + 1, 4 * CH], FP32, tag="osb", name="osb_all")
            gens2 = [attention_gen(g, osb_all) for g in q_local]

            def pump1():
                for gg in list(gens2):
                    try:
                        next(gg)
                        return
                    except StopIteration:
                        gens2.remove(gg)

            for c in range(n_kv):
                kv_ps = ps_pr.tile([128, CH], FP32, tag="prj", name="kv_ps")
                for eb in range(8):
                    nc.tensor.matmul(
                        kv_ps[:], wa_sb[:, 128 * eb : 128 * (eb + 1)],
                        kv_sb[:, CH * eb : CH * (eb + 1)],
                        start=(eb == 0), stop=(eb == 7),
                    )
                vt4 = ps_vt.tile([128, 4 * D], BF16, tag=vt_tag, name="vt4")
                for j in range(4):
                    nc.tensor.transpose(
                        vt4[:, D * j : D * (j + 1)],
                        kv_sb[0:64, CH * c + NB * j : CH * c + NB * (j + 1)],
                        ident[0:64, 0:64],
                    )
                for _ in range(5):
                    pump1()
            while gens2:
                pump1()
            return
        if stage == "mm5":
            # singles like mm2, but round-robin across 8 one-bank psum tiles:
            # WAW distance 8 instead of 1
            pst = [tc.tile([128, CH], FP32, space="PSUM", name=f"p8_{i}")
                   for i in range(8)]
            for c in range(n_kv):
                for eb in range(8):
                    nc.tensor.matmul(
                        pst[eb][:],
                        wa_sb[:, 128 * eb : 128 * (eb + 1)],
                        kv_sb[:, CH * eb : CH * (eb + 1)],
                        start=True, stop=True,
                    )
            return
        if stage == "mm6":
            # 8-chains like fast 'mm', but every chain into a different tile
            pst = [tc.tile([128, CH], FP32, space="PSUM", name=f"p6_{i}")
                   for i in range(4)]
            for c in range(n_kv):
                for eb in range(8):
                    nc.tensor.matmul(
                        pst[c % 4][:],
                        wa_sb[:, 128 * eb : 128 * (eb + 1)],
                        kv_sb[:, CH * eb : CH * (eb + 1)],
                        start=(eb == 0), stop=(eb == 7),
                    )
            return
        if stage in ("mm1", "mm2", "mm3", "mm4"):
            # mutate the fast 'mm' bench one property at a time toward 'sco':
            # mm1: rhs = fixed qtd slice; mm2: singles (start=stop=True);
            # mm3: out = slices of a 2-bank tile; mm4: lhsT = kv_sb slices
            for c in range(n_kv):
                if stage == "mm3":
                    kv_ps = ps_sp.tile([128, 2 * CH], FP32, tag="sp", name="sp")
                else:
                    kv_ps = ps_pr.tile([128, CH], FP32, tag="prj", name="kv_ps")
                for eb in range(8):
                    lhsT = (kv_sb[:, 128 * eb : 128 * (eb + 1)] if stage == "mm4"
                            else wa_sb[:, 128 * eb : 128 * (eb + 1)])
                    rhs = (qtd[0:128, 0:CH] if stage == "mm1"
                           else kv_sb[:, CH * eb : CH * (eb + 1)])
                    out_ = (kv_ps[:, CH * (eb % 2) : CH * (eb % 2 + 1)]
                            if stage == "mm3" else kv_ps[:])
                    single = stage in ("mm2", "mm3")
                    nc.tensor.matmul(
                        out_, lhsT, rhs,
                        start=(True if single else eb == 0),
                        stop=(True if single else eb == 7),
                    )
            return
        if stage in ("scoK", "scoE", "scoC", "scoS"):
            # scoK: K=128 singles; scoE: K=64 all-even-half singles;
            # scoC: K=64 alternating, chained pairs; scoS: K=64 stop-only-sing
            sps = [tc.tile([128, 2 * CH], FP32, space="PSUM", name=f"spq{i}")
                   for i in range(2)]
            for g in range(36):
                sp = sps[g % 2]
                if stage == "scoK":
                    for s in range(2):
                        blk = (2 * g + s) % (4 * n_kv)
                        nc.tensor.matmul(
                            sp[:, CH * s : CH * (s + 1)],
                            kv_sb[0:128, NB * blk : NB * (blk + 1)],
                            qtd[0:128, 0:CH],
                            start=True, stop=True,
                        )
                elif stage == "scoE":
                    for s in range(2):
                        blk = (2 * g + s) % (4 * n_kv)
                        nc.tensor.matmul(
                            sp[:, CH * s : CH * (s + 1)],
                            kv_sb[0:64, NB * blk : NB * (blk + 1)],
                            qtd[0:64, 0:CH],
                            start=True, stop=True,
                        )
                elif stage == "scoC":
                    for s in range(2):
                        half = 64 * s
                        blk = (2 * g + s) % (4 * n_kv)
                        nc.tensor.matmul(
                            sp[:, 0:CH],
                            kv_sb[half : half + 64, NB * blk : NB * (blk + 1)],
                            qtd[half : half + 64, 0:CH],
                            start=(s == 0), stop=(s == 1),
                        )
                elif stage == "scoS":
                    for s in range(2):
                        half = 64 * s
                        blk = (2 * g + s) % (4 * n_kv)
                        nc.tensor.matmul(
                            sp[:, CH * s : CH * (s + 1)],
                            kv_sb[half : half + 64, NB * blk : NB * (blk + 1)],
                            qtd[half : half + 64, 0:CH],
                            start=True, stop=True,
                            tile_position=(half, 0),
                        )
            return
        if stage in ("sco2", "se2", "sea2"):
            # like sco/se/sea but with persistent psum/sbuf tiles reused
            # round-robin instead of per-group pool allocations
            sps = [tc.tile([128, 2 * CH], FP32, space="PSUM", name=f"spp{i}")
                   for i in range(2)]
            etp = [tc.tile([128, 2 * CH], BF16, name=f"etp{i}") for i in range(3)]
            accp = tc.tile([D + 1, CH], FP32, space="PSUM", name="accp")
            n_units = 72
            done = 0
            for g in range(36):
                sp = sps[g % 2]
                for s in range(2):
                    half = 64 * s
                    blk = (2 * g + s) % (4 * n_kv)
                    nc.tensor.matmul(
                        sp[:, CH * s : CH * (s + 1)],
                        kv_sb[half : half + 64, NB * blk : NB * (blk + 1)],
                        qtd[half : half + 64, 0:CH],
                        start=True, stop=True,
                    )
                if stage == "sco2":
                    continue
                e_t = etp[g % 3]
                nc.scalar.activation(
                    e_t[:], sp[:], mybir.ActivationFunctionType.Exp,
                    bias=0.0, scale=SCALE,
                )
                if stage == "se2":
                    continue
                for s in range(2):
                    blk = (2 * g + s) % (4 * n_kv)
                    nc.tensor.matmul(
                        accp[:],
                        v_sb[:, (D + 1) * blk : (D + 1) * (blk + 1)],
                        e_t[:, CH * s : CH * (s + 1)],
                        start=(done == 0), stop=(done == n_units - 1),
                    )
                    done += 1
            return
        if stage in ("sco", "se", "sea"):
            # scores only / +exp / +AV, 36 pair-groups, no masks/epilogue
            acc = ps_acc.tile([D + 1, CH], FP32, tag="acc", name="acc")
            n_units = 72
            done = 0
            for g in range(36):
                sp = ps_sp.tile([128, 2 * CH], FP32, tag="sp", name="sp")
                for s in range(2):
                    half = 64 * s
                    blk = (2 * g + s) % (4 * n_kv)
                    nc.tensor.matmul(
                        sp[:, CH * s : CH * (s + 1)],
                        kv_sb[half : half + 64, NB * blk : NB * (blk + 1)],
                        qtd[half : half + 64, 0:CH],
                        start=True, stop=True,
                    )
                if stage == "sco":
                    continue
                e_t = exp_pool.tile([128, 2 * CH], BF16, tag="et", name="e_t")
                nc.scalar.activation(
                    e_t[:], sp[:], mybir.ActivationFunctionType.Exp,
                    bias=0.0, scale=SCALE,
                )
                if stage == "se":
                    continue
                for s in range(2):
                    blk = (2 * g + s) % (4 * n_kv)
                    nc.tensor.matmul(
                        acc[:],
                        v_sb[:, (D + 1) * blk : (D + 1) * (blk + 1)],
                        e_t[:, CH * s : CH * (s + 1)],
                        start=(done == 0), stop=(done == n_units - 1),
                    )
                    done += 1
            if stage == "sco":
                # keep sp tiles "consumed" so releases are valid
                pass
            return

    MICRO_STAGES = ("empty", "mm", "mmt", "exp", "att", "attmmt", "sco", "se", "sea",
                    "sco2", "se2", "sea2", "scoK", "scoE", "scoC", "scoS",
                    "mm1", "mm2", "mm3", "mm4", "mm5", "mm6")

    def one_rep():
        if stage in MICRO_STAGES:
            return one_rep_micro()
        # input chunks arrive in dma_group-sized grouped DMAs issued upfront
        # (the ~2us fixed cost per dma_start does not overlap on a ring, so
        # fewer/bigger transfers win); nothing ever blocks the input stream.
        xcs = []
        rings = [nc.sync, nc.scalar, nc.gpsimd][: max(1, dma_rings)]
        if not dma_split:
            rings = [nc.sync]
        for gi, g0 in enumerate(range(0, n_kv, dma_group)):
            g = min(dma_group, n_kv - g0)
            xt_grp = xc_pool.tile([128, g * 8 * CH], BF16, tag="xc", name="xc")
            src = xT[128 * g0 : 128 * (g0 + g), :].rearrange("(g p) m -> p g m", p=128)
            rings[gi % len(rings)].dma_start(
                xt_grp[:].rearrange("p (g m) -> p g m", g=g), src
            )
            for i in range(g):
                xcs.append(xt_grp[:, 8 * CH * i : 8 * CH * (i + 1)])
        if stage == "dma":
            return
        osb_all = sm_pool.tile([D + 1, 4 * CH], FP32, tag="osb", name="osb_all")

        from collections import deque

        gens = deque()

        def pump(n):
            for _ in range(n):
                if not gens:
                    return
                try:
                    next(gens[0])
                except StopIteration:
                    gens.popleft()

        for c in range(n_kv):
            xchunk = xcs[c]
            wstack = wa_sb if c % 2 == 0 else wb_sb
            vhalf = 64 * (1 - (c % 2))  # partition base of vT in kv psum

            kv_ps = ps_pr.tile([128, CH], FP32, tag="prj", name="kv_ps")
            for eb in range(8):
                nc.tensor.matmul(
                    kv_ps[:],
                    wstack[:, 128 * eb : 128 * (eb + 1)],
                    xchunk[:, CH * eb : CH * (eb + 1)],
                    start=(eb == 0),
                    stop=(eb == 7),
                )
            # folded kT/vT for this chunk -> persistent kv_sb (single copy)
            nc.vector.tensor_copy(kv_sb[:, CH * c : CH * (c + 1)], kv_ps[:])
            pump(pump_n)

            # v blocks: PE-transpose the four [64,128] vT blocks into ONE
            # single-bank bf16 psum tile (shares banks with the acc pool),
            # evacuated by a single strided DVE copy.
            vt4 = ps_vt.tile([128, 4 * D], BF16, tag=vt_tag, name="vt4")
            for j in range(4):
                nc.tensor.transpose(
                    vt4[:, D * j : D * (j + 1)],
                    kv_sb[vhalf : vhalf + 64, CH * c + NB * j : CH * c + NB * (j + 1)],
                    ident[vhalf : vhalf + 64, vhalf : vhalf + 64],
                )
            v_dst = v_sb[:].rearrange("p (b c) -> p b c", c=D + 1)[
                :, 4 * c : 4 * (c + 1), 0:D
            ]
            nc.vector.tensor_copy(v_dst, vt4[:].rearrange("p (b c) -> p b c", c=D))
            pump(pump_n)

            if c in q_local:
                qi = q_local[c]
                q_ps = ps_pr.tile([128, CH], FP32, tag="prj", name="q_ps")
                for eb in range(8):
                    nc.tensor.matmul(
                        q_ps[:],
                        wq_sb[:, 128 * eb : 128 * (eb + 1)],
                        xchunk[:, CH * eb : CH * (eb + 1)],
                        start=(eb == 0),
                        stop=(eb == 7),
                    )
                nc.vector.tensor_copy(qtd[:, CH * qi : CH * (qi + 1)], q_ps[:])
                if stage not in ("proj",):
                    gens.append(attention_gen(c, osb_all))
            pump(pump_n)
        while gens:
            pump(100)
        if stage not in ("proj",) and out_combined:
            nc.scalar.dma_start(
                out[:].rearrange("(q p) m -> p q m", p=D + 1),
                osb_all[:].rearrange("p (q m) -> p q m", q=len(q_gids)),
            )

    if hw_loop and repeat > 1:
        u = unroll if repeat % unroll == 0 else 1
        with tc.For_i(0, repeat // u, 1):
            for _ in range(u):
                one_rep()
    else:
        for _rep in range(repeat):
            one_rep()


def build_program(n_kv, q_gids, num_devices=4, repeat=1, hw_loop=False, stage="full",
                  **knobs):
    import contextlib

    nc = bacc.Bacc(
        "TRN2", target_bir_lowering=False, debug=False, num_devices=num_devices
    )
    xT = nc.dram_tensor("xT", [n_kv * 128, 8 * CH], BF16, kind="ExternalInput").ap()
    wa = nc.dram_tensor("wa", [128, E], BF16, kind="ExternalInput").ap()
    wb = nc.dram_tensor("wb", [128, E], BF16, kind="ExternalInput").ap()
    wq2 = nc.dram_tensor("wq2", [128, E], BF16, kind="ExternalInput").ap()
    out = nc.dram_tensor(
        "out", [len(q_gids) * (D + 1), CH], FP32, kind="ExternalOutput"
    ).ap()
    with tile.TileContext(nc) as tc:
        with contextlib.ExitStack() as ctx:
            _build_body(ctx, tc, xT, wa, wb, wq2, out, n_kv, q_gids,
                        repeat=repeat, hw_loop=hw_loop, stage=stage, **knobs)
    nc.compile()
    return nc


# ---------------- host-side runner ----------------


def _make_runner(nc, devices, donate=True):
    import jax
    from jax.experimental.shard_map import shard_map
    from jax.sharding import Mesh, PartitionSpec

    from concourse import bass2jax

    bass2jax.install_neuronx_cc_hook()

    fn0 = nc.m.functions[0]
    partition_name = nc.partition_id_tensor.name if nc.partition_id_tensor else None
    in_names, out_names, out_avals = [], [], []
    for alloc in fn0.allocations:
        if not isinstance(alloc, mybir.MemoryLocationSet):
            continue
        if alloc.kind not in ("ExternalInput", "ExternalOutput"):
            continue
        name = alloc.memorylocations[0].name
        if alloc.kind == "ExternalInput":
            if name != partition_name:
                in_names.append(name)
        else:
            out_names.append(name)
            out_avals.append(
                jax.core.ShapedArray(
                    tuple(alloc.tensor_shape), mybir.dt.np(alloc.dtype)
                )
            )
    n_params = len(in_names)
    n_outs = len(out_names)
    all_names = list(in_names) + list(out_names)
    if partition_name is not None:
        all_names.append(partition_name)
    all_names = tuple(all_names)

    def _body(*args):
        operands = list(args)
        if partition_name is not None:
            operands.append(bass2jax.partition_id_tensor())
        outs = bass2jax._bass_exec_p.bind(
            *operands,
            out_avals=tuple(out_avals),
            in_names=all_names,
            out_names=tuple(out_names),
            lowering_input_output_aliases=(),
            sim_require_finite=True,
            sim_require_nnan=True,
            nc=nc,
        )
        return tuple(outs)

    n_cores = len(devices)
    mesh = Mesh(np.asarray(devices), ("core",))
    in_specs = (PartitionSpec("core"),) * (n_params + n_outs)
    out_specs = (PartitionSpec("core"),) * n_outs
    donate_idx = tuple(range(n_params, n_params + n_outs)) if donate else ()
    sharded = jax.jit(
        shard_map(
            _body, mesh=mesh, in_specs=in_specs, out_specs=out_specs, check_rep=False
        ),
        donate_argnums=donate_idx,
        keep_unused=True,
    )
    return {
        "fn": sharded,
        "in_names": in_names,
        "out_names": out_names,
        "out_avals": out_avals,
        "n_cores": n_cores,
        "nc": nc,
        "devices": devices,
    }


@functools.lru_cache(maxsize=1)
def _get_programs():
    import jax

    devs = jax.devices()
    assert len(devs) >= 8, f"need 8 neuron cores, have {devs}"
    nc_outer = build_program(OUTER_NKV, OUTER_GIDS)
    nc_middle = build_program(MIDDLE_NKV, MIDDLE_GIDS)
    run_outer = _make_runner(nc_outer, devs[0:4])
    run_middle = _make_runner(nc_middle, devs[4:8])
    return run_outer, run_middle


def _concat_inputs(runner, per_core_maps):
    arrs = []
    for name in runner["in_names"]:
        arrs.append(np.concatenate([m[name] for m in per_core_maps], axis=0))
    for av in runner["out_avals"]:
        arrs.append(np.zeros((runner["n_cores"] * av.shape[0], *av.shape[1:]), av.dtype))
    return arrs


def _split_outputs(runner, out_arrs):
    res = []
    for c in range(runner["n_cores"]):
        m = {}
        for i, name in enumerate(runner["out_names"]):
            shp = runner["out_avals"][i].shape
            m[name] = np.asarray(out_arrs[i]).reshape(
                runner["n_cores"], *shp
            )[c]
        res.append(m)
    return res


def _bf16(a):
    import ml_dtypes

    return np.asarray(a, dtype=ml_dtypes.bfloat16)


def pack_x(xb, n_kv):
    """Pack x rows [0:512*n_kv) of one batch into the chunk-major DMA layout:
    out[c*128 + p, eb*512 + t] = xb[512*c + t, 128*eb + p]  (bf16)."""
    arr = xb[: CH * n_kv].reshape(n_kv, CH, 8, 128)
    return np.ascontiguousarray(
        _bf16(arr.transpose(0, 3, 2, 1).reshape(n_kv * 128, 8 * CH))
    )


def stack_w(w1, w2):
    """[128, 8*128] bf16: cols [128*eb : 128*eb+64] = w1 block eb, rest w2."""
    a = w1.reshape(8, 128, D).transpose(1, 0, 2)  # [128, 8, 64]
    b = w2.reshape(8, 128, D).transpose(1, 0, 2)
    return np.ascontiguousarray(
        _bf16(np.concatenate([a, b], axis=2).reshape(128, 8 * 128))
    )


def make_core_inputs(x, Wq, Wk, Wv):
    x = np.asarray(x, dtype=np.float32)
    Wq = np.asarray(Wq, dtype=np.float32)
    Wk = np.asarray(Wk, dtype=np.float32)
    Wv = np.asarray(Wv, dtype=np.float32)
    wa = stack_w(Wk, Wv)
    wb = stack_w(Wv, Wk)
    wq2 = stack_w(Wq, Wq)
    outer_maps, middle_maps = [], []
    for b in range(B):
        outer_maps.append(
            {"xT": pack_x(x[b], OUTER_NKV), "wa": wa, "wb": wb, "wq2": wq2}
        )
        middle_maps.append(
            {"xT": pack_x(x[b], MIDDLE_NKV), "wa": wa, "wb": wb, "wq2": wq2}
        )
    return outer_maps, middle_maps


def assemble_output(outer_res, middle_res):
    out = np.empty((B, T, D), dtype=np.float32)
    for b in range(B):
        for res, gids in ((outer_res, OUTER_GIDS), (middle_res, MIDDLE_GIDS)):
            oc = res[b]["out"]  # [4*65, 512] = unnormalized outT per q-chunk
            for qi, g in enumerate(gids):
                blk = oc[(D + 1) * qi : (D + 1) * (qi + 1)]
                out[b, CH * g : CH * (g + 1)] = (blk[0:D] / blk[D : D + 1]).T
    return out


def kernel(x, Wq, Wk, Wv):
    run_outer, run_middle = _get_programs()
    outer_maps, middle_maps = make_core_inputs(x, Wq, Wk, Wv)
    a_in = _concat_inputs(run_outer, outer_maps)
    b_in = _concat_inputs(run_middle, middle_maps)
    a_out = run_outer["fn"](*a_in)  # async dispatch
    b_out = run_middle["fn"](*b_in)
    outer_res = _split_outputs(run_outer, a_out)
    middle_res = _split_outputs(run_middle, b_out)
    return assemble_output(outer_res, middle_res)


if __name__ == "__main__":
    rng = np.random.default_rng(0)
    x = rng.standard_normal((B, T, E), dtype=np.float32)
    s = 1.0 / np.sqrt(E)
    Wq = rng.uniform(-s, s, (E, D)).astype(np.float32)
    Wk = rng.uniform(-s, s, (E, D)).astype(np.float32)
    Wv = rng.uniform(-s, s, (E, D)).astype(np.float32)
    out = kernel(x, Wq, Wk, Wv)
    print("out", out.shape, out.dtype, np.abs(out).mean())


# revision 17
# speedup vs baseline: 1.0793x; 1.0793x over previous
"""Causal single-head attention (B=4, T=4096, E=1024, D=64) on 8 trn2 NeuronCores.

Strategy (bf16 rework of the fp32r baseline; ~1.5-1.6x faster):
  - 2 cores per batch, causally-balanced q split: "outer" core owns q-chunks
    {0,1,6,7} (needs kv chunks 0..7), "middle" owns {2,3,4,5} (kv 0..5).
    Both do 72 score/AV k-blocks.
  - All matmul operands bf16 (x packed to bf16 on host): halves DMA and SBUF
    traffic; PSUM accumulation stays fp32.
  - Natural chunk stream 0,1,2,...: when chunk G (a q-chunk) arrives, all kv
    chunks <= G are already on-chip, so q-chunk G's attention runs immediately
    and completely -> at most one live AV accumulator (+1 draining).
  - Scores computed transposed (S^T[k, q]) with k-chunks folded by parity onto
    partition halves (even chunks' kT on partitions 0:64, odd on 64:128, via
    host-stacked [Wk|Wv] / [Wv|Wk] weights), so score matmuls (K=64) run
    pairwise-concurrent via PE row tiling.
  - Score pairs land in a 2-bank fp32 PSUM tile [128, 1024]; ONE ACT exp
    instruction converts the whole group to bf16 e_t (amortizes the ~300ns
    ACT instruction overhead).
  - Softmax denominator = 65th "ones" column of v in the AV matmul; the
    kernel ships the unnormalized outT [65, 512] per q-chunk (row 64 = the
    denominator) and the host does the divide + layout transpose.
  - Diagonal-block causal masks multiply on the otherwise-idle GPSIMD engine
    so the DVE queue never serializes behind the ACT exp chain.

Two programs (outer/middle) run 4-core SPMD on disjoint device sets.
"""

import functools

import numpy as np

import concourse.bass as bass
import concourse.mybir as mybir
import concourse.tile as tile
from concourse import bacc
from concourse.masks import make_identity

E = 1024
D = 64
T = 4096
B = 4
CH = 512  # 512-row x/q/kv chunk
NB = 128  # k block size (PE partition dim of score output)
SCALE = 1.0 / 32.0  # E ** -0.5

OUTER_GIDS = (0, 1, 6, 7)
MIDDLE_GIDS = (2, 3, 4, 5)
OUTER_NKV = 8
MIDDLE_NKV = 6

FP32 = mybir.dt.float32
FP32R = mybir.dt.float32r
BF16 = mybir.dt.bfloat16


def _build_body(ctx, tc, xT, wa, wb, wq2, out, n_kv, q_gids, repeat=1,
                hw_loop=False, stage="full", dma_split=False, sp_banks=2,
                sp_bufs=None, dma_group=1, prj_bufs=2, vt_share=True,
                out_combined=False, pump_n=2, av_lag=2, dma_rings=2,
                acc_bufs=2, vt_pool="acc", unroll=8, xc_bufs=None, et_extra=1):
    nc = tc.nc
    q_local = {g: i for i, g in enumerate(q_gids)}
    if sp_bufs is None:
        sp_bufs = 4 // sp_banks

    pers = ctx.enter_context(tc.tile_pool(name="pers", bufs=1))
    n_xgrp = -(-n_kv // dma_group)
    xc_pool = ctx.enter_context(
        tc.tile_pool(name="xc", bufs=xc_bufs or max(2, n_xgrp))
    )
    exp_pool = ctx.enter_context(
        tc.tile_pool(name="expp", bufs=sp_bufs + et_extra)
    )
    sm_pool = ctx.enter_context(tc.tile_pool(name="sm", bufs=2))
    ps_sp = ctx.enter_context(tc.tile_pool(name="ps_sp", bufs=sp_bufs, space="PSUM"))
    ps_acc = ctx.enter_context(tc.tile_pool(name="ps_acc", bufs=acc_bufs, space="PSUM"))
    ps_pr = ctx.enter_context(tc.tile_pool(name="ps_pr", bufs=prj_bufs, space="PSUM"))
    if vt_pool == "acc" and vt_share:
        ps_vt, vt_tag = ps_acc, "acc"
    elif vt_pool == "prj":
        ps_vt, vt_tag = ps_pr, "prj"
    else:
        ps_vt = ctx.enter_context(tc.tile_pool(name="ps_vt", bufs=1, space="PSUM"))
        vt_tag = "vt4"

    # ---- persistent SBUF tensors ----
    wa_sb = pers.tile([128, E], BF16, tag="wa")   # [Wk|Wv] per e-block (even chunks)
    wb_sb = pers.tile([128, E], BF16, tag="wb")   # [Wv|Wk] per e-block (odd chunks)
    wq_sb = pers.tile([128, E], BF16, tag="wq2")  # [Wq|Wq] per e-block
    kv_sb = pers.tile([128, n_kv * CH], BF16, tag="kvsb")  # folded kT/vT per chunk
    qtd = pers.tile([128, len(q_gids) * CH], BF16, tag="qtd")  # dup'd qT per q-chunk
    v_sb = pers.tile([128, 4 * n_kv * (D + 1)], BF16, tag="vsb")  # [v | 1] blocks
    masks = pers.tile([128, 4 * CH], BF16, tag="masks")
    ident = pers.tile([128, 128], BF16, tag="ident")
    ones65 = pers.tile([1, D + 1], FP32R, tag="ones65")

    make_identity(nc, ident[:])

    # staircase causal masks M_j[r, c] = 1 iff c - r - 128*j >= 0
    nc.gpsimd.memset(masks[:], 1.0)
    for j in range(4):
        nc.gpsimd.affine_select(
            out=masks[:, CH * j : CH * (j + 1)],
            in_=masks[:, CH * j : CH * (j + 1)],
            compare_op=mybir.AluOpType.is_ge,
            fill=0.0,
            base=-NB * j,
            channel_multiplier=-1,
            pattern=[[1, CH]],
        )

    # v ones-columns + ones row for the reciprocal broadcast matmul
    n_blocks = 4 * n_kv
    ones_view = v_sb[:].rearrange("p (b c) -> p b c", c=D + 1)[:, :, D : D + 1]
    const1 = nc.const_aps.tensor(1.0, (128, n_blocks, 1), FP32)
    nc.scalar.activation(
        ones_view, const1, mybir.ActivationFunctionType.Copy, bias=0.0, scale=1.0
    )
    const1r = nc.const_aps.tensor(1.0, (1, D + 1), FP32)
    nc.scalar.activation(
        ones65[:], const1r, mybir.ActivationFunctionType.Copy, bias=0.0, scale=1.0
    )

    # weights arrive pre-stacked from host: [128, 8*128] bf16
    nc.sync.dma_start(wa_sb[:], wa)
    nc.sync.dma_start(wb_sb[:], wb)
    nc.sync.dma_start(wq_sb[:], wq2)

    if stage not in ("full", "dma", "proj", "noexp", "empty"):  # microbenches
        nc.gpsimd.memset(kv_sb[:], 0.001)
        nc.gpsimd.memset(qtd[:], 0.001)
        nc.gpsimd.memset(v_sb[:], 0.001)

    def epilogue(G, acc, osb_all):
        # stash the unnormalized outT [65, 512] (row 64 = softmax denominator);
        # the host divides + transposes.
        qi = q_local[G]
        if out_combined:
            # one combined out-DMA ships all four at iteration end
            nc.vector.tensor_copy(osb_all[:, CH * qi : CH * (qi + 1)], acc[:])
        else:
            osb = sm_pool.tile([D + 1, CH], FP32, tag="osb1")
            nc.vector.tensor_copy(osb[:], acc[:])
            nc.scalar.dma_start(out[(D + 1) * qi : (D + 1) * (qi + 1), :], osb[:])

    def attention_gen(G, osb_all, lag=None):
        """Full attention for q-chunk G (all kv chunks <= G are on-chip).
        Generator: yields after each score-group emission so the caller can
        interleave projection work into the PE stream (the in-order PE FIFO
        otherwise stalls on the scores->exp->AV chain)."""
        from collections import deque

        qi = q_local[G]
        qcols = slice(CH * qi, CH * (qi + 1))
        evens = [c for c in range(G + 1) if c % 2 == 0]
        odds = [c for c in range(G + 1) if c % 2 == 1]
        # groups: [(ce, co)] pairs then leftover singles, x4 j-blocks each;
        # groups touching the diagonal chunk G go FIRST so the GPSIMD mask
        # latency hides in pipeline fill instead of gating the last AVs
        groups = []
        for ce, co in zip(evens, odds):
            for j in range(4):
                groups.append(((ce, j), (co, j)))
        for c in evens[len(odds):] + odds[len(evens):]:
            for j in range(4):
                groups.append(((c, j),))

        if lag is None:
            lag = av_lag
        acc = ps_acc.tile([D + 1, CH], FP32, tag="acc", name="acc")
        n_units = 4 * (G + 1)
        done = 0
        prevq = deque()

        def flush_one():
            nonlocal done
            ets, units, qofs = prevq.popleft()
            for et, (c, j), o in zip(ets, units, qofs):
                blk = 4 * c + j
                # partial-width AV writes acc cols [o:CH]; safe because the
                # first AV of the chunk (start=True) is always full-width
                nc.tensor.matmul(
                    acc[:, o:CH],
                    v_sb[:, (D + 1) * blk : (D + 1) * (blk + 1)],
                    et,
                    start=(done == 0),
                    stop=(done == n_units - 1),
                )
                done += 1

        for units in groups:
            # diagonal blocks (c == G): q-cols [0, 128j) are entirely below
            # the causal mask -- skip them (scores, exp, AV all narrower).
            qofs = [NB * j if c == G else 0 for c, j in units]
            ws = [CH - o for o in qofs]
            sofs = [sum(ws[:s]) for s in range(len(units))]
            tot = sum(ws)
            sp = ps_sp.tile([128, 2 * CH], FP32, tag="sp", name="sp")
            e_t = exp_pool.tile([128, 2 * CH], BF16, tag="et", name="e_t")
            sps = [sp[:, sofs[s] : sofs[s] + ws[s]] for s in range(len(units))]
            ets = [e_t[:, sofs[s] : sofs[s] + ws[s]] for s in range(len(units))]
            for s, (c, j) in enumerate(units):
                half = 64 * (c % 2)
                nc.tensor.matmul(
                    sps[s],
                    kv_sb[half : half + 64, CH * c + NB * j : CH * c + NB * (j + 1)],
                    qtd[half : half + 64, CH * qi + qofs[s] : CH * (qi + 1)],
                    start=True,
                    stop=True,
                )
            if stage == "noexp":
                nc.vector.tensor_copy(e_t[:, 0:tot], sp[:, 0:tot])
            else:
                nc.scalar.activation(
                    e_t[:, 0:tot], sp[:, 0:tot],
                    mybir.ActivationFunctionType.Exp, bias=0.0, scale=SCALE,
                )
            for s, (c, j) in enumerate(units):
                if c == G:  # partial causal mask (on idle GPSIMD so the DVE
                    # queue never stalls behind the exp chain)
                    nc.gpsimd.tensor_mul(
                        ets[s], ets[s],
                        masks[:, CH * j + qofs[s] : CH * (j + 1)],
                    )
            prevq.append((ets, units, qofs))
            if len(prevq) > lag:
                flush_one()
            yield
        while prevq:
            flush_one()
        epilogue(G, acc, osb_all)
        yield

    def one_rep_micro():
        if stage == "empty":
            t = sm_pool.tile([128, 1], FP32, tag="osb")
            nc.vector.tensor_copy(t[:], masks[:, 0:1])
            return
        if stage == "mm":  # proj-like PE stream, no DMA/copy deps
            for c in range(n_kv):
                kv_ps = ps_pr.tile([128, CH], FP32, tag="prj", name="kv_ps")
                for eb in range(8):
                    nc.tensor.matmul(
                        kv_ps[:], wa_sb[:, 128 * eb : 128 * (eb + 1)],
                        kv_sb[:, CH * eb : CH * (eb + 1)] if n_kv >= 8 else kv_sb[:, 0:CH],
                        start=(eb == 0), stop=(eb == 7),
                    )
            return
        if stage == "mmt":  # proj MMs + transposes
            for c in range(n_kv):
                kv_ps = ps_pr.tile([128, CH], FP32, tag="prj", name="kv_ps")
                for eb in range(8):
                    nc.tensor.matmul(
                        kv_ps[:], wa_sb[:, 128 * eb : 128 * (eb + 1)],
                        kv_sb[:, CH * eb : CH * (eb + 1)] if n_kv >= 8 else kv_sb[:, 0:CH],
                        start=(eb == 0), stop=(eb == 7),
                    )
                vt4 = ps_vt.tile([128, 4 * D], BF16, tag=vt_tag, name="vt4")
                for j in range(4):
                    nc.tensor.transpose(
                        vt4[:, D * j : D * (j + 1)],
                        kv_sb[0:64, CH * c + NB * j : CH * c + NB * (j + 1)],
                        ident[0:64, 0:64],
                    )
            return
        if stage == "exp":  # ACT-only: 36 exps of [128, 1024] from SBUF
            for g in range(36):
                e_t = exp_pool.tile([128, 2 * CH], BF16, tag="et", name="e_t")
                nc.scalar.activation(
                    e_t[:], kv_sb[:, 0 : 2 * CH],
                    mybir.ActivationFunctionType.Exp, bias=0.0, scale=SCALE,
                )
            return
        if stage == "att":  # attention only (kv_sb/qtd hold garbage)
            osb_all = sm_pool.tile([D + 1, 4 * CH], FP32, tag="osb", name="osb_all")
            for g in list(q_local):
                for _ in attention_gen(g, osb_all):
                    pass
            nc.scalar.dma_start(
                out[:].rearrange("(q p) m -> p q m", p=D + 1),
                osb_all[:].rearrange("p (q m) -> p q m", q=len(q_gids)),
            )
            return
        if stage == "attmmt":  # attention + dependency-free proj-like PE work
            osb_all = sm_pool.tile([D + 1, 4 * CH], FP32, tag="osb", name="osb_all")
            gens2 = [attention_gen(g, osb_all) for g in q_local]

            def pump1():
                for gg in list(gens2):
                    try:
                        next(gg)
                        return
                    except StopIteration:
                        gens2.remove(gg)

            for c in range(n_kv):
                kv_ps = ps_pr.tile([128, CH], FP32, tag="prj", name="kv_ps")
                for eb in range(8):
                    nc.tensor.matmul(
                        kv_ps[:], wa_sb[:, 128 * eb : 128 * (eb + 1)],
                        kv_sb[:, CH * eb : CH * (eb + 1)],
                        start=(eb == 0), stop=(eb == 7),
                    )
                vt4 = ps_vt.tile([128, 4 * D], BF16, tag=vt_tag, name="vt4")
                for j in range(4):
                    nc.tensor.transpose(
                        vt4[:, D * j : D * (j + 1)],
                        kv_sb[0:64, CH * c + NB * j : CH * c + NB * (j + 1)],
                        ident[0:64, 0:64],
                    )
                for _ in range(5):
                    pump1()
            while gens2:
                pump1()
            return
        if stage == "mm5":
            # singles like mm2, but round-robin across 8 one-bank psum tiles:
            # WAW distance 8 instead of 1
            pst = [tc.tile([128, CH], FP32, space="PSUM", name=f"p8_{i}")
                   for i in range(8)]
            for c in range(n_kv):
                for eb in range(8):
                    nc.tensor.matmul(
                        pst[eb][:],
                        wa_sb[:, 128 * eb : 128 * (eb + 1)],
                        kv_sb[:, CH * eb : CH * (eb + 1)],
                        start=True, stop=True,
                    )
            return
        if stage == "mm6":
            # 8-chains like fast 'mm', but every chain into a different tile
            pst = [tc.tile([128, CH], FP32, space="PSUM", name=f"p6_{i}")
                   for i in range(4)]
            for c in range(n_kv):
                for eb in range(8):
                    nc.tensor.matmul(
                        pst[c % 4][:],
                        wa_sb[:, 128 * eb : 128 * (eb + 1)],
                        kv_sb[:, CH * eb : CH * (eb + 1)],
                        start=(eb == 0), stop=(eb == 7),
                    )
            return
        if stage in ("mm1", "mm2", "mm3", "mm4"):
            # mutate the fast 'mm' bench one property at a time toward 'sco':
            # mm1: rhs = fixed qtd slice; mm2: singles (start=stop=True);
            # mm3: out = slices of a 2-bank tile; mm4: lhsT = kv_sb slices
            for c in range(n_kv):
                if stage == "mm3":
                    kv_ps = ps_sp.tile([128, 2 * CH], FP32, tag="sp", name="sp")
                else:
                    kv_ps = ps_pr.tile([128, CH], FP32, tag="prj", name="kv_ps")
                for eb in range(8):
                    lhsT = (kv_sb[:, 128 * eb : 128 * (eb + 1)] if stage == "mm4"
                            else wa_sb[:, 128 * eb : 128 * (eb + 1)])
                    rhs = (qtd[0:128, 0:CH] if stage == "mm1"
                           else kv_sb[:, CH * eb : CH * (eb + 1)])
                    out_ = (kv_ps[:, CH * (eb % 2) : CH * (eb % 2 + 1)]
                            if stage == "mm3" else kv_ps[:])
                    single = stage in ("mm2", "mm3")
                    nc.tensor.matmul(
                        out_, lhsT, rhs,
                        start=(True if single else eb == 0),
                        stop=(True if single else eb == 7),
                    )
            return
        if stage in ("scoK", "scoE", "scoC", "scoS"):
            # scoK: K=128 singles; scoE: K=64 all-even-half singles;
            # scoC: K=64 alternating, chained pairs; scoS: K=64 stop-only-sing
            sps = [tc.tile([128, 2 * CH], FP32, space="PSUM", name=f"spq{i}")
                   for i in range(2)]
            for g in range(36):
                sp = sps[g % 2]
                if stage == "scoK":
                    for s in range(2):
                        blk = (2 * g + s) % (4 * n_kv)
                        nc.tensor.matmul(
                            sp[:, CH * s : CH * (s + 1)],
                            kv_sb[0:128, NB * blk : NB * (blk + 1)],
                            qtd[0:128, 0:CH],
                            start=True, stop=True,
                        )
                elif stage == "scoE":
                    for s in range(2):
                        blk = (2 * g + s) % (4 * n_kv)
                        nc.tensor.matmul(
                            sp[:, CH * s : CH * (s + 1)],
                            kv_sb[0:64, NB * blk : NB * (blk + 1)],
                            qtd[0:64, 0:CH],
                            start=True, stop=True,
                        )
                elif stage == "scoC":
                    for s in range(2):
                        half = 64 * s
                        blk = (2 * g + s) % (4 * n_kv)
                        nc.tensor.matmul(
                            sp[:, 0:CH],
                            kv_sb[half : half + 64, NB * blk : NB * (blk + 1)],
                            qtd[half : half + 64, 0:CH],
                            start=(s == 0), stop=(s == 1),
                        )
                elif stage == "scoS":
                    for s in range(2):
                        half = 64 * s
                        blk = (2 * g + s) % (4 * n_kv)
                        nc.tensor.matmul(
                            sp[:, CH * s : CH * (s + 1)],
                            kv_sb[half : half + 64, NB * blk : NB * (blk + 1)],
                            qtd[half : half + 64, 0:CH],
                            start=True, stop=True,
                            tile_position=(half, 0),
                        )
            return
        if stage in ("sco2", "se2", "sea2"):
            # like sco/se/sea but with persistent psum/sbuf tiles reused
            # round-robin instead of per-group pool allocations
            sps = [tc.tile([128, 2 * CH], FP32, space="PSUM", name=f"spp{i}")
                   for i in range(2)]
            etp = [tc.tile([128, 2 * CH], BF16, name=f"etp{i}") for i in range(3)]
            accp = tc.tile([D + 1, CH], FP32, space="PSUM", name="accp")
            n_units = 72
            done = 0
            for g in range(36):
                sp = sps[g % 2]
                for s in range(2):
                    half = 64 * s
                    blk = (2 * g + s) % (4 * n_kv)
                    nc.tensor.matmul(
                        sp[:, CH * s : CH * (s + 1)],
                        kv_sb[half : half + 64, NB * blk : NB * (blk + 1)],
                        qtd[half : half + 64, 0:CH],
                        start=True, stop=True,
                    )
                if stage == "sco2":
                    continue
                e_t = etp[g % 3]
                nc.scalar.activation(
                    e_t[:], sp[:], mybir.ActivationFunctionType.Exp,
                    bias=0.0, scale=SCALE,
                )
                if stage == "se2":
                    continue
                for s in range(2):
                    blk = (2 * g + s) % (4 * n_kv)
                    nc.tensor.matmul(
                        accp[:],
                        v_sb[:, (D + 1) * blk : (D + 1) * (blk + 1)],
                        e_t[:, CH * s : CH * (s + 1)],
                        start=(done == 0), stop=(done == n_units - 1),
                    )
                    done += 1
            return
        if stage in ("sco", "se", "sea"):
            # scores only / +exp / +AV, 36 pair-groups, no masks/epilogue
            acc = ps_acc.tile([D + 1, CH], FP32, tag="acc", name="acc")
            n_units = 72
            done = 0
            for g in range(36):
                sp = ps_sp.tile([128, 2 * CH], FP32, tag="sp", name="sp")
                for s in range(2):
                    half = 64 * s
                    blk = (2 * g + s) % (4 * n_kv)
                    nc.tensor.matmul(
                        sp[:, CH * s : CH * (s + 1)],
                        kv_sb[half : half + 64, NB * blk : NB * (blk + 1)],
                        qtd[half : half + 64, 0:CH],
                        start=True, stop=True,
                    )
                if stage == "sco":
                    continue
                e_t = exp_pool.tile([128, 2 * CH], BF16, tag="et", name="e_t")
                nc.scalar.activation(
                    e_t[:], sp[:], mybir.ActivationFunctionType.Exp,
                    bias=0.0, scale=SCALE,
                )
                if stage == "se":
                    continue
                for s in range(2):
                    blk = (2 * g + s) % (4 * n_kv)
                    nc.tensor.matmul(
                        acc[:],
                        v_sb[:, (D + 1) * blk : (D + 1) * (blk + 1)],
                        e_t[:, CH * s : CH * (s + 1)],
                        start=(done == 0), stop=(done == n_units - 1),
                    )
                    done += 1
            if stage == "sco":
                # keep sp tiles "consumed" so releases are valid
                pass
            return

    MICRO_STAGES = ("empty", "mm", "mmt", "exp", "att", "attmmt", "sco", "se", "sea",
                    "sco2", "se2", "sea2", "scoK", "scoE", "scoC", "scoS",
                    "mm1", "mm2", "mm3", "mm4", "mm5", "mm6")

    def one_rep():
        if stage in MICRO_STAGES:
            return one_rep_micro()
        # input chunks arrive in dma_group-sized grouped DMAs issued upfront
        # (the ~2us fixed cost per dma_start does not overlap on a ring, so
        # fewer/bigger transfers win); nothing ever blocks the input stream.
        xcs = []
        rings = [nc.sync, nc.scalar, nc.gpsimd][: max(1, dma_rings)]
        if not dma_split:
            rings = [nc.sync]
        for gi, g0 in enumerate(range(0, n_kv, dma_group)):
            g = min(dma_group, n_kv - g0)
            xt_grp = xc_pool.tile([128, g * 8 * CH], BF16, tag="xc", name="xc")
            src = xT[128 * g0 : 128 * (g0 + g), :].rearrange("(g p) m -> p g m", p=128)
            rings[gi % len(rings)].dma_start(
                xt_grp[:].rearrange("p (g m) -> p g m", g=g), src
            )
            for i in range(g):
                xcs.append(xt_grp[:, 8 * CH * i : 8 * CH * (i + 1)])
        if stage == "dma":
            return
        osb_all = sm_pool.tile([D + 1, 4 * CH], FP32, tag="osb", name="osb_all")

        from collections import deque

        gens = deque()

        def pump(n):
            for _ in range(n):
                if not gens:
                    return
                try:
                    next(gens[0])
                except StopIteration:
                    gens.popleft()

        for c in range(n_kv):
            xchunk = xcs[c]
            wstack = wa_sb if c % 2 == 0 else wb_sb
            vhalf = 64 * (1 - (c % 2))  # partition base of vT in kv psum

            kv_ps = ps_pr.tile([128, CH], FP32, tag="prj", name="kv_ps")
            for eb in range(8):
                nc.tensor.matmul(
                    kv_ps[:],
                    wstack[:, 128 * eb : 128 * (eb + 1)],
                    xchunk[:, CH * eb : CH * (eb + 1)],
                    start=(eb == 0),
                    stop=(eb == 7),
                )
            # folded kT/vT for this chunk -> persistent kv_sb (single copy)
            nc.vector.tensor_copy(kv_sb[:, CH * c : CH * (c + 1)], kv_ps[:])
            pump(pump_n)

            # v blocks: PE-transpose the four [64,128] vT blocks into ONE
            # single-bank bf16 psum tile (shares banks with the acc pool),
            # evacuated by a single strided DVE copy.
            vt4 = ps_vt.tile([128, 4 * D], BF16, tag=vt_tag, name="vt4")
            for j in range(4):
                nc.tensor.transpose(
                    vt4[:, D * j : D * (j + 1)],
                    kv_sb[vhalf : vhalf + 64, CH * c + NB * j : CH * c + NB * (j + 1)],
                    ident[vhalf : vhalf + 64, vhalf : vhalf + 64],
                )
            v_dst = v_sb[:].rearrange("p (b c) -> p b c", c=D + 1)[
                :, 4 * c : 4 * (c + 1), 0:D
            ]
            nc.vector.tensor_copy(v_dst, vt4[:].rearrange("p (b c) -> p b c", c=D))
            pump(pump_n)

            if c in q_local:
                qi = q_local[c]
                q_ps = ps_pr.tile([128, CH], FP32, tag="prj", name="q_ps")
                for eb in range(8):
                    nc.tensor.matmul(
                        q_ps[:],
                        wq_sb[:, 128 * eb : 128 * (eb + 1)],
                        xchunk[:, CH * eb : CH * (eb + 1)],
                        start=(eb == 0),
                        stop=(eb == 7),
                    )
                nc.vector.tensor_copy(qtd[:, CH * qi : CH * (qi + 1)], q_ps[:])
                if stage not in ("proj",):
                    gens.append(attention_gen(c, osb_all))
            pump(pump_n)
        while gens:
            pump(100)
        if stage not in ("proj",) and out_combined:
            nc.scalar.dma_start(
                out[:].rearrange("(q p) m -> p q m", p=D + 1),
                osb_all[:].rearrange("p (q m) -> p q m", q=len(q_gids)),
            )

    if hw_loop and repeat > 1:
        u = unroll if repeat % unroll == 0 else 1
        with tc.For_i(0, repeat // u, 1):
            for _ in range(u):
                one_rep()
    else:
        for _rep in range(repeat):
            one_rep()


def build_program(n_kv, q_gids, num_devices=4, repeat=1, hw_loop=False, stage="full",
                  **knobs):
    import contextlib

    nc = bacc.Bacc(
        "TRN2", target_bir_lowering=False, debug=False, num_devices=num_devices
    )
    xT = nc.dram_tensor("xT", [n_kv * 128, 8 * CH], BF16, kind="ExternalInput").ap()
    wa = nc.dram_tensor("wa", [128, E], BF16, kind="ExternalInput").ap()
    wb = nc.dram_tensor("wb", [128, E], BF16, kind="ExternalInput").ap()
    wq2 = nc.dram_tensor("wq2", [128, E], BF16, kind="ExternalInput").ap()
    out = nc.dram_tensor(
        "out", [len(q_gids) * (D + 1), CH], FP32, kind="ExternalOutput"
    ).ap()
    with tile.TileContext(nc) as tc:
        with contextlib.ExitStack() as ctx:
            _build_body(ctx, tc, xT, wa, wb, wq2, out, n_kv, q_gids,
                        repeat=repeat, hw_loop=hw_loop, stage=stage, **knobs)
    nc.compile()
    return nc


# ---------------- host-side runner ----------------


def _make_runner(nc, devices, donate=True):
    import jax
    from jax.experimental.shard_map import shard_map
    from jax.sharding import Mesh, PartitionSpec

    from concourse import bass2jax

    bass2jax.install_neuronx_cc_hook()

    fn0 = nc.m.functions[0]
    partition_name = nc.partition_id_tensor.name if nc.partition_id_tensor else None
    in_names, out_names, out_avals = [], [], []
    for alloc in fn0.allocations:
        if not isinstance(alloc, mybir.MemoryLocationSet):
            continue
        if alloc.kind not in ("ExternalInput", "ExternalOutput"):
            continue
        name = alloc.memorylocations[0].name
        if alloc.kind == "ExternalInput":
            if name != partition_name:
                in_names.append(name)
        else:
            out_names.append(name)
            out_avals.append(
                jax.core.ShapedArray(
                    tuple(alloc.tensor_shape), mybir.dt.np(alloc.dtype)
                )
            )
    n_params = len(in_names)
    n_outs = len(out_names)
    all_names = list(in_names) + list(out_names)
    if partition_name is not None:
        all_names.append(partition_name)
    all_names = tuple(all_names)

    def _body(*args):
        operands = list(args)
        if partition_name is not None:
            operands.append(bass2jax.partition_id_tensor())
        outs = bass2jax._bass_exec_p.bind(
            *operands,
            out_avals=tuple(out_avals),
            in_names=all_names,
            out_names=tuple(out_names),
            lowering_input_output_aliases=(),
            sim_require_finite=True,
            sim_require_nnan=True,
            nc=nc,
        )
        return tuple(outs)

    n_cores = len(devices)
    mesh = Mesh(np.asarray(devices), ("core",))
    in_specs = (PartitionSpec("core"),) * (n_params + n_outs)
    out_specs = (PartitionSpec("core"),) * n_outs
    donate_idx = tuple(range(n_params, n_params + n_outs)) if donate else ()
    sharded = jax.jit(
        shard_map(
            _body, mesh=mesh, in_specs=in_specs, out_specs=out_specs, check_rep=False
        ),
        donate_argnums=donate_idx,
        keep_unused=True,
    )
    return {
        "fn": sharded,
        "in_names": in_names,
        "out_names": out_names,
        "out_avals": out_avals,
        "n_cores": n_cores,
        "nc": nc,
        "devices": devices,
    }


@functools.lru_cache(maxsize=1)
def _get_programs():
    import jax

    devs = jax.devices()
    assert len(devs) >= 8, f"need 8 neuron cores, have {devs}"
    nc_outer = build_program(OUTER_NKV, OUTER_GIDS)
    nc_middle = build_program(MIDDLE_NKV, MIDDLE_GIDS)
    run_outer = _make_runner(nc_outer, devs[0:4])
    run_middle = _make_runner(nc_middle, devs[4:8])
    return run_outer, run_middle


def _concat_inputs(runner, per_core_maps):
    arrs = []
    for name in runner["in_names"]:
        arrs.append(np.concatenate([m[name] for m in per_core_maps], axis=0))
    for av in runner["out_avals"]:
        arrs.append(np.zeros((runner["n_cores"] * av.shape[0], *av.shape[1:]), av.dtype))
    return arrs


def _split_outputs(runner, out_arrs):
    res = []
    for c in range(runner["n_cores"]):
        m = {}
        for i, name in enumerate(runner["out_names"]):
            shp = runner["out_avals"][i].shape
            m[name] = np.asarray(out_arrs[i]).reshape(
                runner["n_cores"], *shp
            )[c]
        res.append(m)
    return res


def _bf16(a):
    import ml_dtypes

    return np.asarray(a, dtype=ml_dtypes.bfloat16)


def pack_x(xb, n_kv):
    """Pack x rows [0:512*n_kv) of one batch into the chunk-major DMA layout:
    out[c*128 + p, eb*512 + t] = xb[512*c + t, 128*eb + p]  (bf16)."""
    arr = xb[: CH * n_kv].reshape(n_kv, CH, 8, 128)
    return np.ascontiguousarray(
        _bf16(arr.transpose(0, 3, 2, 1).reshape(n_kv * 128, 8 * CH))
    )


def stack_w(w1, w2):
    """[128, 8*128] bf16: cols [128*eb : 128*eb+64] = w1 block eb, rest w2."""
    a = w1.reshape(8, 128, D).transpose(1, 0, 2)  # [128, 8, 64]
    b = w2.reshape(8, 128, D).transpose(1, 0, 2)
    return np.ascontiguousarray(
        _bf16(np.concatenate([a, b], axis=2).reshape(128, 8 * 128))
    )


def make_core_inputs(x, Wq, Wk, Wv):
    x = np.asarray(x, dtype=np.float32)
    Wq = np.asarray(Wq, dtype=np.float32)
    Wk = np.asarray(Wk, dtype=np.float32)
    Wv = np.asarray(Wv, dtype=np.float32)
    wa = stack_w(Wk, Wv)
    wb = stack_w(Wv, Wk)
    wq2 = stack_w(Wq, Wq)
    outer_maps, middle_maps = [], []
    for b in range(B):
        outer_maps.append(
            {"xT": pack_x(x[b], OUTER_NKV), "wa": wa, "wb": wb, "wq2": wq2}
        )
        middle_maps.append(
            {"xT": pack_x(x[b], MIDDLE_NKV), "wa": wa, "wb": wb, "wq2": wq2}
        )
    return outer_maps, middle_maps


def assemble_output(outer_res, middle_res):
    out = np.empty((B, T, D), dtype=np.float32)
    for b in range(B):
        for res, gids in ((outer_res, OUTER_GIDS), (middle_res, MIDDLE_GIDS)):
            oc = res[b]["out"]  # [4*65, 512] = unnormalized outT per q-chunk
            for qi, g in enumerate(gids):
                blk = oc[(D + 1) * qi : (D + 1) * (qi + 1)]
                out[b, CH * g : CH * (g + 1)] = (blk[0:D] / blk[D : D + 1]).T
    return out


def kernel(x, Wq, Wk, Wv):
    run_outer, run_middle = _get_programs()
    outer_maps, middle_maps = make_core_inputs(x, Wq, Wk, Wv)
    a_in = _concat_inputs(run_outer, outer_maps)
    b_in = _concat_inputs(run_middle, middle_maps)
    a_out = run_outer["fn"](*a_in)  # async dispatch
    b_out = run_middle["fn"](*b_in)
    outer_res = _split_outputs(run_outer, a_out)
    middle_res = _split_outputs(run_middle, b_out)
    return assemble_output(outer_res, middle_res)


if __name__ == "__main__":
    rng = np.random.default_rng(0)
    x = rng.standard_normal((B, T, E), dtype=np.float32)
    s = 1.0 / np.sqrt(E)
    Wq = rng.uniform(-s, s, (E, D)).astype(np.float32)
    Wk = rng.uniform(-s, s, (E, D)).astype(np.float32)
    Wv = rng.uniform(-s, s, (E, D)).astype(np.float32)
    out = kernel(x, Wq, Wk, Wv)
    print("out", out.shape, out.dtype, np.abs(out).mean())


# revision 18
# speedup vs baseline: 1.0806x; 1.0013x over previous
"""Causal single-head attention (B=4, T=4096, E=1024, D=64) on 8 trn2 NeuronCores.

Strategy (bf16 rework of the fp32r baseline; ~1.5-1.6x faster):
  - 2 cores per batch, causally-balanced q split: "outer" core owns q-chunks
    {0,1,6,7} (needs kv chunks 0..7), "middle" owns {2,3,4,5} (kv 0..5).
    Both do 72 score/AV k-blocks.
  - All matmul operands bf16 (x packed to bf16 on host): halves DMA and SBUF
    traffic; PSUM accumulation stays fp32.
  - Natural chunk stream 0,1,2,...: when chunk G (a q-chunk) arrives, all kv
    chunks <= G are already on-chip, so q-chunk G's attention runs immediately
    and completely -> at most one live AV accumulator (+1 draining).
  - Scores computed transposed (S^T[k, q]) with k-chunks folded by parity onto
    partition halves (even chunks' kT on partitions 0:64, odd on 64:128, via
    host-stacked [Wk|Wv] / [Wv|Wk] weights), so score matmuls (K=64) run
    pairwise-concurrent via PE row tiling.
  - Score pairs land in a 2-bank fp32 PSUM tile [128, 1024]; ONE ACT exp
    instruction converts the whole group to bf16 e_t (amortizes the ~300ns
    ACT instruction overhead).
  - Softmax denominator = 65th "ones" column of v in the AV matmul; the
    kernel ships the unnormalized outT [65, 512] per q-chunk (row 64 = the
    denominator) and the host does the divide + layout transpose.
  - Diagonal-block causal masks multiply on the otherwise-idle GPSIMD engine
    so the DVE queue never serializes behind the ACT exp chain.

Two programs (outer/middle) run 4-core SPMD on disjoint device sets.
"""

import functools

import numpy as np

import concourse.bass as bass
import concourse.mybir as mybir
import concourse.tile as tile
from concourse import bacc
from concourse.masks import make_identity

E = 1024
D = 64
T = 4096
B = 4
CH = 512  # 512-row x/q/kv chunk
NB = 128  # k block size (PE partition dim of score output)
SCALE = 1.0 / 32.0  # E ** -0.5

OUTER_GIDS = (0, 1, 6, 7)
MIDDLE_GIDS = (2, 3, 4, 5)
OUTER_NKV = 8
MIDDLE_NKV = 6

FP32 = mybir.dt.float32
FP32R = mybir.dt.float32r
BF16 = mybir.dt.bfloat16


def _build_body(ctx, tc, xT, wa, wb, wq2, out, n_kv, q_gids, repeat=1,
                hw_loop=False, stage="full", dma_split=False, sp_banks=2,
                sp_bufs=None, dma_group=1, prj_bufs=2, vt_share=True,
                out_combined=False, pump_n=2, av_lag=2, dma_rings=2,
                acc_bufs=2, vt_pool="acc", unroll=8, xc_bufs=None, et_extra=1,
                out_ring="gpsimd"):
    nc = tc.nc
    q_local = {g: i for i, g in enumerate(q_gids)}
    if sp_bufs is None:
        sp_bufs = 4 // sp_banks

    pers = ctx.enter_context(tc.tile_pool(name="pers", bufs=1))
    n_xgrp = -(-n_kv // dma_group)
    xc_pool = ctx.enter_context(
        tc.tile_pool(name="xc", bufs=xc_bufs or max(2, n_xgrp))
    )
    exp_pool = ctx.enter_context(
        tc.tile_pool(name="expp", bufs=sp_bufs + et_extra)
    )
    sm_pool = ctx.enter_context(tc.tile_pool(name="sm", bufs=2))
    ps_sp = ctx.enter_context(tc.tile_pool(name="ps_sp", bufs=sp_bufs, space="PSUM"))
    ps_acc = ctx.enter_context(tc.tile_pool(name="ps_acc", bufs=acc_bufs, space="PSUM"))
    ps_pr = ctx.enter_context(tc.tile_pool(name="ps_pr", bufs=prj_bufs, space="PSUM"))
    if vt_pool == "acc" and vt_share:
        ps_vt, vt_tag = ps_acc, "acc"
    elif vt_pool == "prj":
        ps_vt, vt_tag = ps_pr, "prj"
    else:
        ps_vt = ctx.enter_context(tc.tile_pool(name="ps_vt", bufs=1, space="PSUM"))
        vt_tag = "vt4"

    # ---- persistent SBUF tensors ----
    wa_sb = pers.tile([128, E], BF16, tag="wa")   # [Wk|Wv] per e-block (even chunks)
    wb_sb = pers.tile([128, E], BF16, tag="wb")   # [Wv|Wk] per e-block (odd chunks)
    wq_sb = pers.tile([128, E], BF16, tag="wq2")  # [Wq|Wq] per e-block
    kv_sb = pers.tile([128, n_kv * CH], BF16, tag="kvsb")  # folded kT/vT per chunk
    qtd = pers.tile([128, len(q_gids) * CH], BF16, tag="qtd")  # dup'd qT per q-chunk
    v_sb = pers.tile([128, 4 * n_kv * (D + 1)], BF16, tag="vsb")  # [v | 1] blocks
    masks = pers.tile([128, 4 * CH], BF16, tag="masks")
    ident = pers.tile([128, 128], BF16, tag="ident")
    ones65 = pers.tile([1, D + 1], FP32R, tag="ones65")

    make_identity(nc, ident[:])

    # staircase causal masks M_j[r, c] = 1 iff c - r - 128*j >= 0
    nc.gpsimd.memset(masks[:], 1.0)
    for j in range(4):
        nc.gpsimd.affine_select(
            out=masks[:, CH * j : CH * (j + 1)],
            in_=masks[:, CH * j : CH * (j + 1)],
            compare_op=mybir.AluOpType.is_ge,
            fill=0.0,
            base=-NB * j,
            channel_multiplier=-1,
            pattern=[[1, CH]],
        )

    # v ones-columns + ones row for the reciprocal broadcast matmul
    n_blocks = 4 * n_kv
    ones_view = v_sb[:].rearrange("p (b c) -> p b c", c=D + 1)[:, :, D : D + 1]
    const1 = nc.const_aps.tensor(1.0, (128, n_blocks, 1), FP32)
    nc.scalar.activation(
        ones_view, const1, mybir.ActivationFunctionType.Copy, bias=0.0, scale=1.0
    )
    const1r = nc.const_aps.tensor(1.0, (1, D + 1), FP32)
    nc.scalar.activation(
        ones65[:], const1r, mybir.ActivationFunctionType.Copy, bias=0.0, scale=1.0
    )

    # weights arrive pre-stacked from host: [128, 8*128] bf16
    nc.sync.dma_start(wa_sb[:], wa)
    nc.sync.dma_start(wb_sb[:], wb)
    nc.sync.dma_start(wq_sb[:], wq2)

    if stage not in ("full", "dma", "proj", "noexp", "empty"):  # microbenches
        nc.gpsimd.memset(kv_sb[:], 0.001)
        nc.gpsimd.memset(qtd[:], 0.001)
        nc.gpsimd.memset(v_sb[:], 0.001)

    def epilogue(G, acc, osb_all):
        # stash the unnormalized outT [65, 512] (row 64 = softmax denominator);
        # the host divides + transposes.
        qi = q_local[G]
        if out_combined:
            # one combined out-DMA ships all four at iteration end
            nc.vector.tensor_copy(osb_all[:, CH * qi : CH * (qi + 1)], acc[:])
        else:
            osb = sm_pool.tile([D + 1, CH], FP32, tag="osb1")
            nc.vector.tensor_copy(osb[:], acc[:])
            eng = {"scalar": nc.scalar, "gpsimd": nc.gpsimd, "sync": nc.sync}[out_ring]
            eng.dma_start(out[(D + 1) * qi : (D + 1) * (qi + 1), :], osb[:])

    def attention_gen(G, osb_all, lag=None):
        """Full attention for q-chunk G (all kv chunks <= G are on-chip).
        Generator: yields after each score-group emission so the caller can
        interleave projection work into the PE stream (the in-order PE FIFO
        otherwise stalls on the scores->exp->AV chain)."""
        from collections import deque

        qi = q_local[G]
        qcols = slice(CH * qi, CH * (qi + 1))
        evens = [c for c in range(G + 1) if c % 2 == 0]
        odds = [c for c in range(G + 1) if c % 2 == 1]
        # groups: [(ce, co)] pairs then leftover singles, x4 j-blocks each;
        # groups touching the diagonal chunk G go FIRST so the GPSIMD mask
        # latency hides in pipeline fill instead of gating the last AVs
        groups = []
        for ce, co in zip(evens, odds):
            for j in range(4):
                groups.append(((ce, j), (co, j)))
        for c in evens[len(odds):] + odds[len(evens):]:
            for j in range(4):
                groups.append(((c, j),))

        if lag is None:
            lag = av_lag
        acc = ps_acc.tile([D + 1, CH], FP32, tag="acc", name="acc")
        n_units = 4 * (G + 1)
        done = 0
        prevq = deque()

        def flush_one():
            nonlocal done
            ets, units, qofs = prevq.popleft()
            for et, (c, j), o in zip(ets, units, qofs):
                blk = 4 * c + j
                # partial-width AV writes acc cols [o:CH]; safe because the
                # first AV of the chunk (start=True) is always full-width
                nc.tensor.matmul(
                    acc[:, o:CH],
                    v_sb[:, (D + 1) * blk : (D + 1) * (blk + 1)],
                    et,
                    start=(done == 0),
                    stop=(done == n_units - 1),
                )
                done += 1

        for units in groups:
            # diagonal blocks (c == G): q-cols [0, 128j) are entirely below
            # the causal mask -- skip them (scores, exp, AV all narrower).
            qofs = [NB * j if c == G else 0 for c, j in units]
            ws = [CH - o for o in qofs]
            sofs = [sum(ws[:s]) for s in range(len(units))]
            tot = sum(ws)
            sp = ps_sp.tile([128, 2 * CH], FP32, tag="sp", name="sp")
            e_t = exp_pool.tile([128, 2 * CH], BF16, tag="et", name="e_t")
            sps = [sp[:, sofs[s] : sofs[s] + ws[s]] for s in range(len(units))]
            ets = [e_t[:, sofs[s] : sofs[s] + ws[s]] for s in range(len(units))]
            for s, (c, j) in enumerate(units):
                half = 64 * (c % 2)
                nc.tensor.matmul(
                    sps[s],
                    kv_sb[half : half + 64, CH * c + NB * j : CH * c + NB * (j + 1)],
                    qtd[half : half + 64, CH * qi + qofs[s] : CH * (qi + 1)],
                    start=True,
                    stop=True,
                )
            if stage == "noexp":
                nc.vector.tensor_copy(e_t[:, 0:tot], sp[:, 0:tot])
            else:
                nc.scalar.activation(
                    e_t[:, 0:tot], sp[:, 0:tot],
                    mybir.ActivationFunctionType.Exp, bias=0.0, scale=SCALE,
                )
            for s, (c, j) in enumerate(units):
                if c == G:  # partial causal mask (on idle GPSIMD so the DVE
                    # queue never stalls behind the exp chain)
                    nc.gpsimd.tensor_mul(
                        ets[s], ets[s],
                        masks[:, CH * j + qofs[s] : CH * (j + 1)],
                    )
            prevq.append((ets, units, qofs))
            if len(prevq) > lag:
                flush_one()
            yield
        while prevq:
            flush_one()
        epilogue(G, acc, osb_all)
        yield

    def one_rep_micro():
        if stage == "empty":
            t = sm_pool.tile([128, 1], FP32, tag="osb")
            nc.vector.tensor_copy(t[:], masks[:, 0:1])
            return
        if stage == "mm":  # proj-like PE stream, no DMA/copy deps
            for c in range(n_kv):
                kv_ps = ps_pr.tile([128, CH], FP32, tag="prj", name="kv_ps")
                for eb in range(8):
                    nc.tensor.matmul(
                        kv_ps[:], wa_sb[:, 128 * eb : 128 * (eb + 1)],
                        kv_sb[:, CH * eb : CH * (eb + 1)] if n_kv >= 8 else kv_sb[:, 0:CH],
                        start=(eb == 0), stop=(eb == 7),
                    )
            return
        if stage == "mmt":  # proj MMs + transposes
            for c in range(n_kv):
                kv_ps = ps_pr.tile([128, CH], FP32, tag="prj", name="kv_ps")
                for eb in range(8):
                    nc.tensor.matmul(
                        kv_ps[:], wa_sb[:, 128 * eb : 128 * (eb + 1)],
                        kv_sb[:, CH * eb : CH * (eb + 1)] if n_kv >= 8 else kv_sb[:, 0:CH],
                        start=(eb == 0), stop=(eb == 7),
                    )
                vt4 = ps_vt.tile([128, 4 * D], BF16, tag=vt_tag, name="vt4")
                for j in range(4):
                    nc.tensor.transpose(
                        vt4[:, D * j : D * (j + 1)],
                        kv_sb[0:64, CH * c + NB * j : CH * c + NB * (j + 1)],
                        ident[0:64, 0:64],
                    )
            return
        if stage == "exp":  # ACT-only: 36 exps of [128, 1024] from SBUF
            for g in range(36):
                e_t = exp_pool.tile([128, 2 * CH], BF16, tag="et", name="e_t")
                nc.scalar.activation(
                    e_t[:], kv_sb[:, 0 : 2 * CH],
                    mybir.ActivationFunctionType.Exp, bias=0.0, scale=SCALE,
                )
            return
        if stage == "att":  # attention only (kv_sb/qtd hold garbage)
            osb_all = sm_pool.tile([D + 1, 4 * CH], FP32, tag="osb", name="osb_all")
            for g in list(q_local):
                for _ in attention_gen(g, osb_all):
                    pass
            nc.scalar.dma_start(
                out[:].rearrange("(q p) m -> p q m", p=D + 1),
                osb_all[:].rearrange("p (q m) -> p q m", q=len(q_gids)),
            )
            return
        if stage == "attmmt":  # attention + dependency-free proj-like PE work
            osb_all = sm_pool.tile([D + 1, 4 * CH], FP32, tag="osb", name="osb_all")
            gens2 = [attention_gen(g, osb_all) for g in q_local]

            def pump1():
                for gg in list(gens2):
                    try:
                        next(gg)
                        return
                    except StopIteration:
                        gens2.remove(gg)

            for c in range(n_kv):
                kv_ps = ps_pr.tile([128, CH], FP32, tag="prj", name="kv_ps")
                for eb in range(8):
                    nc.tensor.matmul(
                        kv_ps[:], wa_sb[:, 128 * eb : 128 * (eb + 1)],
                        kv_sb[:, CH * eb : CH * (eb + 1)],
                        start=(eb == 0), stop=(eb == 7),
                    )
                vt4 = ps_vt.tile([128, 4 * D], BF16, tag=vt_tag, name="vt4")
                for j in range(4):
                    nc.tensor.transpose(
                        vt4[:, D * j : D * (j + 1)],
                        kv_sb[0:64, CH * c + NB * j : CH * c + NB * (j + 1)],
                        ident[0:64, 0:64],
                    )
                for _ in range(5):
                    pump1()
            while gens2:
                pump1()
            return
        if stage == "mm5":
            # singles like mm2, but round-robin across 8 one-bank psum tiles:
            # WAW distance 8 instead of 1
            pst = [tc.tile([128, CH], FP32, space="PSUM", name=f"p8_{i}")
                   for i in range(8)]
            for c in range(n_kv):
                for eb in range(8):
                    nc.tensor.matmul(
                        pst[eb][:],
                        wa_sb[:, 128 * eb : 128 * (eb + 1)],
                        kv_sb[:, CH * eb : CH * (eb + 1)],
                        start=True, stop=True,
                    )
            return
        if stage == "mm6":
            # 8-chains like fast 'mm', but every chain into a different tile
            pst = [tc.tile([128, CH], FP32, space="PSUM", name=f"p6_{i}")
                   for i in range(4)]
            for c in range(n_kv):
                for eb in range(8):
                    nc.tensor.matmul(
                        pst[c % 4][:],
                        wa_sb[:, 128 * eb : 128 * (eb + 1)],
                        kv_sb[:, CH * eb : CH * (eb + 1)],
                        start=(eb == 0), stop=(eb == 7),
                    )
            return
        if stage in ("mm1", "mm2", "mm3", "mm4"):
            # mutate the fast 'mm' bench one property at a time toward 'sco':
            # mm1: rhs = fixed qtd slice; mm2: singles (start=stop=True);
            # mm3: out = slices of a 2-bank tile; mm4: lhsT = kv_sb slices
            for c in range(n_kv):
                if stage == "mm3":
                    kv_ps = ps_sp.tile([128, 2 * CH], FP32, tag="sp", name="sp")
                else:
                    kv_ps = ps_pr.tile([128, CH], FP32, tag="prj", name="kv_ps")
                for eb in range(8):
                    lhsT = (kv_sb[:, 128 * eb : 128 * (eb + 1)] if stage == "mm4"
                            else wa_sb[:, 128 * eb : 128 * (eb + 1)])
                    rhs = (qtd[0:128, 0:CH] if stage == "mm1"
                           else kv_sb[:, CH * eb : CH * (eb + 1)])
                    out_ = (kv_ps[:, CH * (eb % 2) : CH * (eb % 2 + 1)]
                            if stage == "mm3" else kv_ps[:])
                    single = stage in ("mm2", "mm3")
                    nc.tensor.matmul(
                        out_, lhsT, rhs,
                        start=(True if single else eb == 0),
                        stop=(True if single else eb == 7),
                    )
            return
        if stage in ("scoK", "scoE", "scoC", "scoS"):
            # scoK: K=128 singles; scoE: K=64 all-even-half singles;
            # scoC: K=64 alternating, chained pairs; scoS: K=64 stop-only-sing
            sps = [tc.tile([128, 2 * CH], FP32, space="PSUM", name=f"spq{i}")
                   for i in range(2)]
            for g in range(36):
                sp = sps[g % 2]
                if stage == "scoK":
                    for s in range(2):
                        blk = (2 * g + s) % (4 * n_kv)
                        nc.tensor.matmul(
                            sp[:, CH * s : CH * (s + 1)],
                            kv_sb[0:128, NB * blk : NB * (blk + 1)],
                            qtd[0:128, 0:CH],
                            start=True, stop=True,
                        )
                elif stage == "scoE":
                    for s in range(2):
                        blk = (2 * g + s) % (4 * n_kv)
                        nc.tensor.matmul(
                            sp[:, CH * s : CH * (s + 1)],
                            kv_sb[0:64, NB * blk : NB * (blk + 1)],
                            qtd[0:64, 0:CH],
                            start=True, stop=True,
                        )
                elif stage == "scoC":
                    for s in range(2):
                        half = 64 * s
                        blk = (2 * g + s) % (4 * n_kv)
                        nc.tensor.matmul(
                            sp[:, 0:CH],
                            kv_sb[half : half + 64, NB * blk : NB * (blk + 1)],
                            qtd[half : half + 64, 0:CH],
                            start=(s == 0), stop=(s == 1),
                        )
                elif stage == "scoS":
                    for s in range(2):
                        half = 64 * s
                        blk = (2 * g + s) % (4 * n_kv)
                        nc.tensor.matmul(
                            sp[:, CH * s : CH * (s + 1)],
                            kv_sb[half : half + 64, NB * blk : NB * (blk + 1)],
                            qtd[half : half + 64, 0:CH],
                            start=True, stop=True,
                            tile_position=(half, 0),
                        )
            return
        if stage in ("sco2", "se2", "sea2"):
            # like sco/se/sea but with persistent psum/sbuf tiles reused
            # round-robin instead of per-group pool allocations
            sps = [tc.tile([128, 2 * CH], FP32, space="PSUM", name=f"spp{i}")
                   for i in range(2)]
            etp = [tc.tile([128, 2 * CH], BF16, name=f"etp{i}") for i in range(3)]
            accp = tc.tile([D + 1, CH], FP32, space="PSUM", name="accp")
            n_units = 72
            done = 0
            for g in range(36):
                sp = sps[g % 2]
                for s in range(2):
                    half = 64 * s
                    blk = (2 * g + s) % (4 * n_kv)
                    nc.tensor.matmul(
                        sp[:, CH * s : CH * (s + 1)],
                        kv_sb[half : half + 64, NB * blk : NB * (blk + 1)],
                        qtd[half : half + 64, 0:CH],
                        start=True, stop=True,
                    )
                if stage == "sco2":
                    continue
                e_t = etp[g % 3]
                nc.scalar.activation(
                    e_t[:], sp[:], mybir.ActivationFunctionType.Exp,
                    bias=0.0, scale=SCALE,
                )
                if stage == "se2":
                    continue
                for s in range(2):
                    blk = (2 * g + s) % (4 * n_kv)
                    nc.tensor.matmul(
                        accp[:],
                        v_sb[:, (D + 1) * blk : (D + 1) * (blk + 1)],
                        e_t[:, CH * s : CH * (s + 1)],
                        start=(done == 0), stop=(done == n_units - 1),
                    )
                    done += 1
            return
        if stage in ("sco", "se", "sea"):
            # scores only / +exp / +AV, 36 pair-groups, no masks/epilogue
            acc = ps_acc.tile([D + 1, CH], FP32, tag="acc", name="acc")
            n_units = 72
            done = 0
            for g in range(36):
                sp = ps_sp.tile([128, 2 * CH], FP32, tag="sp", name="sp")
                for s in range(2):
                    half = 64 * s
                    blk = (2 * g + s) % (4 * n_kv)
                    nc.tensor.matmul(
                        sp[:, CH * s : CH * (s + 1)],
                        kv_sb[half : half + 64, NB * blk : NB * (blk + 1)],
                        qtd[half : half + 64, 0:CH],
                        start=True, stop=True,
                    )
                if stage == "sco":
                    continue
                e_t = exp_pool.tile([128, 2 * CH], BF16, tag="et", name="e_t")
                nc.scalar.activation(
                    e_t[:], sp[:], mybir.ActivationFunctionType.Exp,
                    bias=0.0, scale=SCALE,
                )
                if stage == "se":
                    continue
                for s in range(2):
                    blk = (2 * g + s) % (4 * n_kv)
                    nc.tensor.matmul(
                        acc[:],
                        v_sb[:, (D + 1) * blk : (D + 1) * (blk + 1)],
                        e_t[:, CH * s : CH * (s + 1)],
                        start=(done == 0), stop=(done == n_units - 1),
                    )
                    done += 1
            if stage == "sco":
                # keep sp tiles "consumed" so releases are valid
                pass
            return

    MICRO_STAGES = ("empty", "mm", "mmt", "exp", "att", "attmmt", "sco", "se", "sea",
                    "sco2", "se2", "sea2", "scoK", "scoE", "scoC", "scoS",
                    "mm1", "mm2", "mm3", "mm4", "mm5", "mm6")

    def one_rep():
        if stage in MICRO_STAGES:
            return one_rep_micro()
        # input chunks arrive in dma_group-sized grouped DMAs issued upfront
        # (the ~2us fixed cost per dma_start does not overlap on a ring, so
        # fewer/bigger transfers win); nothing ever blocks the input stream.
        xcs = []
        rings = [nc.sync, nc.scalar, nc.gpsimd][: max(1, dma_rings)]
        if not dma_split:
            rings = [nc.sync]
        for gi, g0 in enumerate(range(0, n_kv, dma_group)):
            g = min(dma_group, n_kv - g0)
            xt_grp = xc_pool.tile([128, g * 8 * CH], BF16, tag="xc", name="xc")
            src = xT[128 * g0 : 128 * (g0 + g), :].rearrange("(g p) m -> p g m", p=128)
            rings[gi % len(rings)].dma_start(
                xt_grp[:].rearrange("p (g m) -> p g m", g=g), src
            )
            for i in range(g):
                xcs.append(xt_grp[:, 8 * CH * i : 8 * CH * (i + 1)])
        if stage == "dma":
            return
        osb_all = sm_pool.tile([D + 1, 4 * CH], FP32, tag="osb", name="osb_all")

        from collections import deque

        gens = deque()

        def pump(n):
            for _ in range(n):
                if not gens:
                    return
                try:
                    next(gens[0])
                except StopIteration:
                    gens.popleft()

        for c in range(n_kv):
            xchunk = xcs[c]
            wstack = wa_sb if c % 2 == 0 else wb_sb
            vhalf = 64 * (1 - (c % 2))  # partition base of vT in kv psum

            kv_ps = ps_pr.tile([128, CH], FP32, tag="prj", name="kv_ps")
            for eb in range(8):
                nc.tensor.matmul(
                    kv_ps[:],
                    wstack[:, 128 * eb : 128 * (eb + 1)],
                    xchunk[:, CH * eb : CH * (eb + 1)],
                    start=(eb == 0),
                    stop=(eb == 7),
                )
            # folded kT/vT for this chunk -> persistent kv_sb (single copy)
            nc.vector.tensor_copy(kv_sb[:, CH * c : CH * (c + 1)], kv_ps[:])
            pump(pump_n)

            # v blocks: PE-transpose the four [64,128] vT blocks into ONE
            # single-bank bf16 psum tile (shares banks with the acc pool),
            # evacuated by a single strided DVE copy.
            vt4 = ps_vt.tile([128, 4 * D], BF16, tag=vt_tag, name="vt4")
            for j in range(4):
                nc.tensor.transpose(
                    vt4[:, D * j : D * (j + 1)],
                    kv_sb[vhalf : vhalf + 64, CH * c + NB * j : CH * c + NB * (j + 1)],
                    ident[vhalf : vhalf + 64, vhalf : vhalf + 64],
                )
            v_dst = v_sb[:].rearrange("p (b c) -> p b c", c=D + 1)[
                :, 4 * c : 4 * (c + 1), 0:D
            ]
            nc.vector.tensor_copy(v_dst, vt4[:].rearrange("p (b c) -> p b c", c=D))
            pump(pump_n)

            if c in q_local:
                qi = q_local[c]
                q_ps = ps_pr.tile([128, CH], FP32, tag="prj", name="q_ps")
                for eb in range(8):
                    nc.tensor.matmul(
                        q_ps[:],
                        wq_sb[:, 128 * eb : 128 * (eb + 1)],
                        xchunk[:, CH * eb : CH * (eb + 1)],
                        start=(eb == 0),
                        stop=(eb == 7),
                    )
                nc.vector.tensor_copy(qtd[:, CH * qi : CH * (qi + 1)], q_ps[:])
                if stage not in ("proj",):
                    gens.append(attention_gen(c, osb_all))
            pump(pump_n)
        while gens:
            pump(100)
        if stage not in ("proj",) and out_combined:
            nc.scalar.dma_start(
                out[:].rearrange("(q p) m -> p q m", p=D + 1),
                osb_all[:].rearrange("p (q m) -> p q m", q=len(q_gids)),
            )

    if hw_loop and repeat > 1:
        u = unroll if repeat % unroll == 0 else 1
        with tc.For_i(0, repeat // u, 1):
            for _ in range(u):
                one_rep()
    else:
        for _rep in range(repeat):
            one_rep()


def build_program(n_kv, q_gids, num_devices=4, repeat=1, hw_loop=False, stage="full",
                  **knobs):
    import contextlib

    nc = bacc.Bacc(
        "TRN2", target_bir_lowering=False, debug=False, num_devices=num_devices
    )
    xT = nc.dram_tensor("xT", [n_kv * 128, 8 * CH], BF16, kind="ExternalInput").ap()
    wa = nc.dram_tensor("wa", [128, E], BF16, kind="ExternalInput").ap()
    wb = nc.dram_tensor("wb", [128, E], BF16, kind="ExternalInput").ap()
    wq2 = nc.dram_tensor("wq2", [128, E], BF16, kind="ExternalInput").ap()
    out = nc.dram_tensor(
        "out", [len(q_gids) * (D + 1), CH], FP32, kind="ExternalOutput"
    ).ap()
    with tile.TileContext(nc) as tc:
        with contextlib.ExitStack() as ctx:
            _build_body(ctx, tc, xT, wa, wb, wq2, out, n_kv, q_gids,
                        repeat=repeat, hw_loop=hw_loop, stage=stage, **knobs)
    nc.compile()
    return nc


# ---------------- host-side runner ----------------


def _make_runner(nc, devices, donate=True):
    import jax
    from jax.experimental.shard_map import shard_map
    from jax.sharding import Mesh, PartitionSpec

    from concourse import bass2jax

    bass2jax.install_neuronx_cc_hook()

    fn0 = nc.m.functions[0]
    partition_name = nc.partition_id_tensor.name if nc.partition_id_tensor else None
    in_names, out_names, out_avals = [], [], []
    for alloc in fn0.allocations:
        if not isinstance(alloc, mybir.MemoryLocationSet):
            continue
        if alloc.kind not in ("ExternalInput", "ExternalOutput"):
            continue
        name = alloc.memorylocations[0].name
        if alloc.kind == "ExternalInput":
            if name != partition_name:
                in_names.append(name)
        else:
            out_names.append(name)
            out_avals.append(
                jax.core.ShapedArray(
                    tuple(alloc.tensor_shape), mybir.dt.np(alloc.dtype)
                )
            )
    n_params = len(in_names)
    n_outs = len(out_names)
    all_names = list(in_names) + list(out_names)
    if partition_name is not None:
        all_names.append(partition_name)
    all_names = tuple(all_names)

    def _body(*args):
        operands = list(args)
        if partition_name is not None:
            operands.append(bass2jax.partition_id_tensor())
        outs = bass2jax._bass_exec_p.bind(
            *operands,
            out_avals=tuple(out_avals),
            in_names=all_names,
            out_names=tuple(out_names),
            lowering_input_output_aliases=(),
            sim_require_finite=True,
            sim_require_nnan=True,
            nc=nc,
        )
        return tuple(outs)

    n_cores = len(devices)
    mesh = Mesh(np.asarray(devices), ("core",))
    in_specs = (PartitionSpec("core"),) * (n_params + n_outs)
    out_specs = (PartitionSpec("core"),) * n_outs
    donate_idx = tuple(range(n_params, n_params + n_outs)) if donate else ()
    sharded = jax.jit(
        shard_map(
            _body, mesh=mesh, in_specs=in_specs, out_specs=out_specs, check_rep=False
        ),
        donate_argnums=donate_idx,
        keep_unused=True,
    )
    return {
        "fn": sharded,
        "in_names": in_names,
        "out_names": out_names,
        "out_avals": out_avals,
        "n_cores": n_cores,
        "nc": nc,
        "devices": devices,
    }


@functools.lru_cache(maxsize=1)
def _get_programs():
    import jax

    devs = jax.devices()
    assert len(devs) >= 8, f"need 8 neuron cores, have {devs}"
    nc_outer = build_program(OUTER_NKV, OUTER_GIDS)
    nc_middle = build_program(MIDDLE_NKV, MIDDLE_GIDS)
    run_outer = _make_runner(nc_outer, devs[0:4])
    run_middle = _make_runner(nc_middle, devs[4:8])
    return run_outer, run_middle


def _concat_inputs(runner, per_core_maps):
    arrs = []
    for name in runner["in_names"]:
        arrs.append(np.concatenate([m[name] for m in per_core_maps], axis=0))
    for av in runner["out_avals"]:
        arrs.append(np.zeros((runner["n_cores"] * av.shape[0], *av.shape[1:]), av.dtype))
    return arrs


def _split_outputs(runner, out_arrs):
    res = []
    for c in range(runner["n_cores"]):
        m = {}
        for i, name in enumerate(runner["out_names"]):
            shp = runner["out_avals"][i].shape
            m[name] = np.asarray(out_arrs[i]).reshape(
                runner["n_cores"], *shp
            )[c]
        res.append(m)
    return res


def _bf16(a):
    import ml_dtypes

    return np.asarray(a, dtype=ml_dtypes.bfloat16)


def pack_x(xb, n_kv):
    """Pack x rows [0:512*n_kv) of one batch into the chunk-major DMA layout:
    out[c*128 + p, eb*512 + t] = xb[512*c + t, 128*eb + p]  (bf16)."""
    arr = xb[: CH * n_kv].reshape(n_kv, CH, 8, 128)
    return np.ascontiguousarray(
        _bf16(arr.transpose(0, 3, 2, 1).reshape(n_kv * 128, 8 * CH))
    )


def stack_w(w1, w2):
    """[128, 8*128] bf16: cols [128*eb : 128*eb+64] = w1 block eb, rest w2."""
    a = w1.reshape(8, 128, D).transpose(1, 0, 2)  # [128, 8, 64]
    b = w2.reshape(8, 128, D).transpose(1, 0, 2)
    return np.ascontiguousarray(
        _bf16(np.concatenate([a, b], axis=2).reshape(128, 8 * 128))
    )


def make_core_inputs(x, Wq, Wk, Wv):
    x = np.asarray(x, dtype=np.float32)
    Wq = np.asarray(Wq, dtype=np.float32)
    Wk = np.asarray(Wk, dtype=np.float32)
    Wv = np.asarray(Wv, dtype=np.float32)
    wa = stack_w(Wk, Wv)
    wb = stack_w(Wv, Wk)
    wq2 = stack_w(Wq, Wq)
    outer_maps, middle_maps = [], []
    for b in range(B):
        outer_maps.append(
            {"xT": pack_x(x[b], OUTER_NKV), "wa": wa, "wb": wb, "wq2": wq2}
        )
        middle_maps.append(
            {"xT": pack_x(x[b], MIDDLE_NKV), "wa": wa, "wb": wb, "wq2": wq2}
        )
    return outer_maps, middle_maps


def assemble_output(outer_res, middle_res):
    out = np.empty((B, T, D), dtype=np.float32)
    for b in range(B):
        for res, gids in ((outer_res, OUTER_GIDS), (middle_res, MIDDLE_GIDS)):
            oc = res[b]["out"]  # [4*65, 512] = unnormalized outT per q-chunk
            for qi, g in enumerate(gids):
                blk = oc[(D + 1) * qi : (D + 1) * (qi + 1)]
                out[b, CH * g : CH * (g + 1)] = (blk[0:D] / blk[D : D + 1]).T
    return out


def kernel(x, Wq, Wk, Wv):
    run_outer, run_middle = _get_programs()
    outer_maps, middle_maps = make_core_inputs(x, Wq, Wk, Wv)
    a_in = _concat_inputs(run_outer, outer_maps)
    b_in = _concat_inputs(run_middle, middle_maps)
    a_out = run_outer["fn"](*a_in)  # async dispatch
    b_out = run_middle["fn"](*b_in)
    outer_res = _split_outputs(run_outer, a_out)
    middle_res = _split_outputs(run_middle, b_out)
    return assemble_output(outer_res, middle_res)


if __name__ == "__main__":
    rng = np.random.default_rng(0)
    x = rng.standard_normal((B, T, E), dtype=np.float32)
    s = 1.0 / np.sqrt(E)
    Wq = rng.uniform(-s, s, (E, D)).astype(np.float32)
    Wk = rng.uniform(-s, s, (E, D)).astype(np.float32)
    Wv = rng.uniform(-s, s, (E, D)).astype(np.float32)
    out = kernel(x, Wq, Wk, Wv)
    print("out", out.shape, out.dtype, np.abs(out).mean())


# revision 19
# speedup vs baseline: 1.0892x; 1.0079x over previous
"""Causal single-head attention (B=4, T=4096, E=1024, D=64) on 8 trn2 NeuronCores.

Strategy (bf16 rework of the fp32r baseline; ~1.5-1.6x faster):
  - 2 cores per batch, causally-balanced q split: "outer" core owns q-chunks
    {0,1,6,7} (needs kv chunks 0..7), "middle" owns {2,3,4,5} (kv 0..5).
    Both do 72 score/AV k-blocks.
  - All matmul operands bf16 (x packed to bf16 on host): halves DMA and SBUF
    traffic; PSUM accumulation stays fp32.
  - Natural chunk stream 0,1,2,...: when chunk G (a q-chunk) arrives, all kv
    chunks <= G are already on-chip, so q-chunk G's attention runs immediately
    and completely -> at most one live AV accumulator (+1 draining).
  - Scores computed transposed (S^T[k, q]) with k-chunks folded by parity onto
    partition halves (even chunks' kT on partitions 0:64, odd on 64:128, via
    host-stacked [Wk|Wv] / [Wv|Wk] weights), so score matmuls (K=64) run
    pairwise-concurrent via PE row tiling.
  - Score pairs land in a 2-bank fp32 PSUM tile [128, 1024]; ONE ACT exp
    instruction converts the whole group to bf16 e_t (amortizes the ~300ns
    ACT instruction overhead).
  - Softmax denominator = 65th "ones" column of v in the AV matmul; the
    kernel ships the unnormalized outT [65, 512] per q-chunk (row 64 = the
    denominator) and the host does the divide + layout transpose.
  - Diagonal-block causal masks multiply on the otherwise-idle GPSIMD engine
    so the DVE queue never serializes behind the ACT exp chain.

Two programs (outer/middle) run 4-core SPMD on disjoint device sets.
"""

import functools

import numpy as np

import concourse.bass as bass
import concourse.mybir as mybir
import concourse.tile as tile
from concourse import bacc
from concourse.masks import make_identity

E = 1024
D = 64
T = 4096
B = 4
CH = 512  # 512-row x/q/kv chunk
NB = 128  # k block size (PE partition dim of score output)
SCALE = 1.0 / 32.0  # E ** -0.5

OUTER_GIDS = (0, 1, 6, 7)
MIDDLE_GIDS = (2, 3, 4, 5)
OUTER_NKV = 8
MIDDLE_NKV = 6

FP32 = mybir.dt.float32
FP32R = mybir.dt.float32r
BF16 = mybir.dt.bfloat16


def _build_body(ctx, tc, xT, wa, wb, wq2, out, n_kv, q_gids, repeat=1,
                hw_loop=False, stage="full", dma_split=False, sp_banks=2,
                sp_bufs=None, dma_group=1, prj_bufs=2, vt_share=True,
                out_combined=False, pump_n=2, av_lag=2, dma_rings=2,
                acc_bufs=2, vt_pool="acc", unroll=16, xc_bufs=None, et_extra=1,
                out_ring="gpsimd"):
    nc = tc.nc
    q_local = {g: i for i, g in enumerate(q_gids)}
    if sp_bufs is None:
        sp_bufs = 4 // sp_banks

    pers = ctx.enter_context(tc.tile_pool(name="pers", bufs=1))
    n_xgrp = -(-n_kv // dma_group)
    xc_pool = ctx.enter_context(
        tc.tile_pool(name="xc", bufs=xc_bufs or max(2, n_xgrp))
    )
    exp_pool = ctx.enter_context(
        tc.tile_pool(name="expp", bufs=sp_bufs + et_extra)
    )
    sm_pool = ctx.enter_context(tc.tile_pool(name="sm", bufs=2))
    ps_sp = ctx.enter_context(tc.tile_pool(name="ps_sp", bufs=sp_bufs, space="PSUM"))
    ps_acc = ctx.enter_context(tc.tile_pool(name="ps_acc", bufs=acc_bufs, space="PSUM"))
    ps_pr = ctx.enter_context(tc.tile_pool(name="ps_pr", bufs=prj_bufs, space="PSUM"))
    if vt_pool == "acc" and vt_share:
        ps_vt, vt_tag = ps_acc, "acc"
    elif vt_pool == "prj":
        ps_vt, vt_tag = ps_pr, "prj"
    else:
        ps_vt = ctx.enter_context(tc.tile_pool(name="ps_vt", bufs=1, space="PSUM"))
        vt_tag = "vt4"

    # ---- persistent SBUF tensors ----
    wa_sb = pers.tile([128, E], BF16, tag="wa")   # [Wk|Wv] per e-block (even chunks)
    wb_sb = pers.tile([128, E], BF16, tag="wb")   # [Wv|Wk] per e-block (odd chunks)
    wq_sb = pers.tile([128, E], BF16, tag="wq2")  # [Wq|Wq] per e-block
    kv_sb = pers.tile([128, n_kv * CH], BF16, tag="kvsb")  # folded kT/vT per chunk
    qtd = pers.tile([128, len(q_gids) * CH], BF16, tag="qtd")  # dup'd qT per q-chunk
    v_sb = pers.tile([128, 4 * n_kv * (D + 1)], BF16, tag="vsb")  # [v | 1] blocks
    masks = pers.tile([128, 4 * CH], BF16, tag="masks")
    ident = pers.tile([128, 128], BF16, tag="ident")
    ones65 = pers.tile([1, D + 1], FP32R, tag="ones65")

    make_identity(nc, ident[:])

    # staircase causal masks M_j[r, c] = 1 iff c - r - 128*j >= 0
    nc.gpsimd.memset(masks[:], 1.0)
    for j in range(4):
        nc.gpsimd.affine_select(
            out=masks[:, CH * j : CH * (j + 1)],
            in_=masks[:, CH * j : CH * (j + 1)],
            compare_op=mybir.AluOpType.is_ge,
            fill=0.0,
            base=-NB * j,
            channel_multiplier=-1,
            pattern=[[1, CH]],
        )

    # v ones-columns + ones row for the reciprocal broadcast matmul
    n_blocks = 4 * n_kv
    ones_view = v_sb[:].rearrange("p (b c) -> p b c", c=D + 1)[:, :, D : D + 1]
    const1 = nc.const_aps.tensor(1.0, (128, n_blocks, 1), FP32)
    nc.scalar.activation(
        ones_view, const1, mybir.ActivationFunctionType.Copy, bias=0.0, scale=1.0
    )
    const1r = nc.const_aps.tensor(1.0, (1, D + 1), FP32)
    nc.scalar.activation(
        ones65[:], const1r, mybir.ActivationFunctionType.Copy, bias=0.0, scale=1.0
    )

    # weights arrive pre-stacked from host: [128, 8*128] bf16
    nc.sync.dma_start(wa_sb[:], wa)
    nc.sync.dma_start(wb_sb[:], wb)
    nc.sync.dma_start(wq_sb[:], wq2)

    if stage not in ("full", "dma", "proj", "noexp", "empty"):  # microbenches
        nc.gpsimd.memset(kv_sb[:], 0.001)
        nc.gpsimd.memset(qtd[:], 0.001)
        nc.gpsimd.memset(v_sb[:], 0.001)

    def epilogue(G, acc, osb_all):
        # stash the unnormalized outT [65, 512] (row 64 = softmax denominator);
        # the host divides + transposes.
        qi = q_local[G]
        if out_combined:
            # one combined out-DMA ships all four at iteration end
            nc.vector.tensor_copy(osb_all[:, CH * qi : CH * (qi + 1)], acc[:])
        else:
            osb = sm_pool.tile([D + 1, CH], FP32, tag="osb1")
            nc.vector.tensor_copy(osb[:], acc[:])
            eng = {"scalar": nc.scalar, "gpsimd": nc.gpsimd, "sync": nc.sync}[out_ring]
            eng.dma_start(out[(D + 1) * qi : (D + 1) * (qi + 1), :], osb[:])

    def attention_gen(G, osb_all, lag=None):
        """Full attention for q-chunk G (all kv chunks <= G are on-chip).
        Generator: yields after each score-group emission so the caller can
        interleave projection work into the PE stream (the in-order PE FIFO
        otherwise stalls on the scores->exp->AV chain)."""
        from collections import deque

        qi = q_local[G]
        qcols = slice(CH * qi, CH * (qi + 1))
        evens = [c for c in range(G + 1) if c % 2 == 0]
        odds = [c for c in range(G + 1) if c % 2 == 1]
        # groups: [(ce, co)] pairs then leftover singles, x4 j-blocks each;
        # groups touching the diagonal chunk G go FIRST so the GPSIMD mask
        # latency hides in pipeline fill instead of gating the last AVs
        groups = []
        for ce, co in zip(evens, odds):
            for j in range(4):
                groups.append(((ce, j), (co, j)))
        for c in evens[len(odds):] + odds[len(evens):]:
            for j in range(4):
                groups.append(((c, j),))

        if lag is None:
            lag = av_lag
        acc = ps_acc.tile([D + 1, CH], FP32, tag="acc", name="acc")
        n_units = 4 * (G + 1)
        done = 0
        prevq = deque()

        def flush_one():
            nonlocal done
            ets, units, qofs = prevq.popleft()
            for et, (c, j), o in zip(ets, units, qofs):
                blk = 4 * c + j
                # partial-width AV writes acc cols [o:CH]; safe because the
                # first AV of the chunk (start=True) is always full-width
                nc.tensor.matmul(
                    acc[:, o:CH],
                    v_sb[:, (D + 1) * blk : (D + 1) * (blk + 1)],
                    et,
                    start=(done == 0),
                    stop=(done == n_units - 1),
                )
                done += 1

        for units in groups:
            # diagonal blocks (c == G): q-cols [0, 128j) are entirely below
            # the causal mask -- skip them (scores, exp, AV all narrower).
            qofs = [NB * j if c == G else 0 for c, j in units]
            ws = [CH - o for o in qofs]
            sofs = [sum(ws[:s]) for s in range(len(units))]
            tot = sum(ws)
            sp = ps_sp.tile([128, 2 * CH], FP32, tag="sp", name="sp")
            e_t = exp_pool.tile([128, 2 * CH], BF16, tag="et", name="e_t")
            sps = [sp[:, sofs[s] : sofs[s] + ws[s]] for s in range(len(units))]
            ets = [e_t[:, sofs[s] : sofs[s] + ws[s]] for s in range(len(units))]
            for s, (c, j) in enumerate(units):
                half = 64 * (c % 2)
                nc.tensor.matmul(
                    sps[s],
                    kv_sb[half : half + 64, CH * c + NB * j : CH * c + NB * (j + 1)],
                    qtd[half : half + 64, CH * qi + qofs[s] : CH * (qi + 1)],
                    start=True,
                    stop=True,
                )
            if stage == "noexp":
                nc.vector.tensor_copy(e_t[:, 0:tot], sp[:, 0:tot])
            else:
                nc.scalar.activation(
                    e_t[:, 0:tot], sp[:, 0:tot],
                    mybir.ActivationFunctionType.Exp, bias=0.0, scale=SCALE,
                )
            for s, (c, j) in enumerate(units):
                if c == G:  # partial causal mask (on idle GPSIMD so the DVE
                    # queue never stalls behind the exp chain)
                    nc.gpsimd.tensor_mul(
                        ets[s], ets[s],
                        masks[:, CH * j + qofs[s] : CH * (j + 1)],
                    )
            prevq.append((ets, units, qofs))
            if len(prevq) > lag:
                flush_one()
            yield
        while prevq:
            flush_one()
        epilogue(G, acc, osb_all)
        yield

    def one_rep_micro():
        if stage == "empty":
            t = sm_pool.tile([128, 1], FP32, tag="osb")
            nc.vector.tensor_copy(t[:], masks[:, 0:1])
            return
        if stage == "mm":  # proj-like PE stream, no DMA/copy deps
            for c in range(n_kv):
                kv_ps = ps_pr.tile([128, CH], FP32, tag="prj", name="kv_ps")
                for eb in range(8):
                    nc.tensor.matmul(
                        kv_ps[:], wa_sb[:, 128 * eb : 128 * (eb + 1)],
                        kv_sb[:, CH * eb : CH * (eb + 1)] if n_kv >= 8 else kv_sb[:, 0:CH],
                        start=(eb == 0), stop=(eb == 7),
                    )
            return
        if stage == "mmt":  # proj MMs + transposes
            for c in range(n_kv):
                kv_ps = ps_pr.tile([128, CH], FP32, tag="prj", name="kv_ps")
                for eb in range(8):
                    nc.tensor.matmul(
                        kv_ps[:], wa_sb[:, 128 * eb : 128 * (eb + 1)],
                        kv_sb[:, CH * eb : CH * (eb + 1)] if n_kv >= 8 else kv_sb[:, 0:CH],
                        start=(eb == 0), stop=(eb == 7),
                    )
                vt4 = ps_vt.tile([128, 4 * D], BF16, tag=vt_tag, name="vt4")
                for j in range(4):
                    nc.tensor.transpose(
                        vt4[:, D * j : D * (j + 1)],
                        kv_sb[0:64, CH * c + NB * j : CH * c + NB * (j + 1)],
                        ident[0:64, 0:64],
                    )
            return
        if stage == "exp":  # ACT-only: 36 exps of [128, 1024] from SBUF
            for g in range(36):
                e_t = exp_pool.tile([128, 2 * CH], BF16, tag="et", name="e_t")
                nc.scalar.activation(
                    e_t[:], kv_sb[:, 0 : 2 * CH],
                    mybir.ActivationFunctionType.Exp, bias=0.0, scale=SCALE,
                )
            return
        if stage == "att":  # attention only (kv_sb/qtd hold garbage)
            osb_all = sm_pool.tile([D + 1, 4 * CH], FP32, tag="osb", name="osb_all")
            for g in list(q_local):
                for _ in attention_gen(g, osb_all):
                    pass
            nc.scalar.dma_start(
                out[:].rearrange("(q p) m -> p q m", p=D + 1),
                osb_all[:].rearrange("p (q m) -> p q m", q=len(q_gids)),
            )
            return
        if stage == "attmmt":  # attention + dependency-free proj-like PE work
            osb_all = sm_pool.tile([D + 1, 4 * CH], FP32, tag="osb", name="osb_all")
            gens2 = [attention_gen(g, osb_all) for g in q_local]

            def pump1():
                for gg in list(gens2):
                    try:
                        next(gg)
                        return
                    except StopIteration:
                        gens2.remove(gg)

            for c in range(n_kv):
                kv_ps = ps_pr.tile([128, CH], FP32, tag="prj", name="kv_ps")
                for eb in range(8):
                    nc.tensor.matmul(
                        kv_ps[:], wa_sb[:, 128 * eb : 128 * (eb + 1)],
                        kv_sb[:, CH * eb : CH * (eb + 1)],
                        start=(eb == 0), stop=(eb == 7),
                    )
                vt4 = ps_vt.tile([128, 4 * D], BF16, tag=vt_tag, name="vt4")
                for j in range(4):
                    nc.tensor.transpose(
                        vt4[:, D * j : D * (j + 1)],
                        kv_sb[0:64, CH * c + NB * j : CH * c + NB * (j + 1)],
                        ident[0:64, 0:64],
                    )
                for _ in range(5):
                    pump1()
            while gens2:
                pump1()
            return
        if stage == "mm5":
            # singles like mm2, but round-robin across 8 one-bank psum tiles:
            # WAW distance 8 instead of 1
            pst = [tc.tile([128, CH], FP32, space="PSUM", name=f"p8_{i}")
                   for i in range(8)]
            for c in range(n_kv):
                for eb in range(8):
                    nc.tensor.matmul(
                        pst[eb][:],
                        wa_sb[:, 128 * eb : 128 * (eb + 1)],
                        kv_sb[:, CH * eb : CH * (eb + 1)],
                        start=True, stop=True,
                    )
            return
        if stage == "mm6":
            # 8-chains like fast 'mm', but every chain into a different tile
            pst = [tc.tile([128, CH], FP32, space="PSUM", name=f"p6_{i}")
                   for i in range(4)]
            for c in range(n_kv):
                for eb in range(8):
                    nc.tensor.matmul(
                        pst[c % 4][:],
                        wa_sb[:, 128 * eb : 128 * (eb + 1)],
                        kv_sb[:, CH * eb : CH * (eb + 1)],
                        start=(eb == 0), stop=(eb == 7),
                    )
            return
        if stage in ("mm1", "mm2", "mm3", "mm4"):
            # mutate the fast 'mm' bench one property at a time toward 'sco':
            # mm1: rhs = fixed qtd slice; mm2: singles (start=stop=True);
            # mm3: out = slices of a 2-bank tile; mm4: lhsT = kv_sb slices
            for c in range(n_kv):
                if stage == "mm3":
                    kv_ps = ps_sp.tile([128, 2 * CH], FP32, tag="sp", name="sp")
                else:
                    kv_ps = ps_pr.tile([128, CH], FP32, tag="prj", name="kv_ps")
                for eb in range(8):
                    lhsT = (kv_sb[:, 128 * eb : 128 * (eb + 1)] if stage == "mm4"
                            else wa_sb[:, 128 * eb : 128 * (eb + 1)])
                    rhs = (qtd[0:128, 0:CH] if stage == "mm1"
                           else kv_sb[:, CH * eb : CH * (eb + 1)])
                    out_ = (kv_ps[:, CH * (eb % 2) : CH * (eb % 2 + 1)]
                            if stage == "mm3" else kv_ps[:])
                    single = stage in ("mm2", "mm3")
                    nc.tensor.matmul(
                        out_, lhsT, rhs,
                        start=(True if single else eb == 0),
                        stop=(True if single else eb == 7),
                    )
            return
        if stage in ("scoK", "scoE", "scoC", "scoS"):
            # scoK: K=128 singles; scoE: K=64 all-even-half singles;
            # scoC: K=64 alternating, chained pairs; scoS: K=64 stop-only-sing
            sps = [tc.tile([128, 2 * CH], FP32, space="PSUM", name=f"spq{i}")
                   for i in range(2)]
            for g in range(36):
                sp = sps[g % 2]
                if stage == "scoK":
                    for s in range(2):
                        blk = (2 * g + s) % (4 * n_kv)
                        nc.tensor.matmul(
                            sp[:, CH * s : CH * (s + 1)],
                            kv_sb[0:128, NB * blk : NB * (blk + 1)],
                            qtd[0:128, 0:CH],
                            start=True, stop=True,
                        )
                elif stage == "scoE":
                    for s in range(2):
                        blk = (2 * g + s) % (4 * n_kv)
                        nc.tensor.matmul(
                            sp[:, CH * s : CH * (s + 1)],
                            kv_sb[0:64, NB * blk : NB * (blk + 1)],
                            qtd[0:64, 0:CH],
                            start=True, stop=True,
                        )
                elif stage == "scoC":
                    for s in range(2):
                        half = 64 * s
                        blk = (2 * g + s) % (4 * n_kv)
                        nc.tensor.matmul(
                            sp[:, 0:CH],
                            kv_sb[half : half + 64, NB * blk : NB * (blk + 1)],
                            qtd[half : half + 64, 0:CH],
                            start=(s == 0), stop=(s == 1),
                        )
                elif stage == "scoS":
                    for s in range(2):
                        half = 64 * s
                        blk = (2 * g + s) % (4 * n_kv)
                        nc.tensor.matmul(
                            sp[:, CH * s : CH * (s + 1)],
                            kv_sb[half : half + 64, NB * blk : NB * (blk + 1)],
                            qtd[half : half + 64, 0:CH],
                            start=True, stop=True,
                            tile_position=(half, 0),
                        )
            return
        if stage in ("sco2", "se2", "sea2"):
            # like sco/se/sea but with persistent psum/sbuf tiles reused
            # round-robin instead of per-group pool allocations
            sps = [tc.tile([128, 2 * CH], FP32, space="PSUM", name=f"spp{i}")
                   for i in range(2)]
            etp = [tc.tile([128, 2 * CH], BF16, name=f"etp{i}") for i in range(3)]
            accp = tc.tile([D + 1, CH], FP32, space="PSUM", name="accp")
            n_units = 72
            done = 0
            for g in range(36):
                sp = sps[g % 2]
                for s in range(2):
                    half = 64 * s
                    blk = (2 * g + s) % (4 * n_kv)
                    nc.tensor.matmul(
                        sp[:, CH * s : CH * (s + 1)],
                        kv_sb[half : half + 64, NB * blk : NB * (blk + 1)],
                        qtd[half : half + 64, 0:CH],
                        start=True, stop=True,
                    )
                if stage == "sco2":
                    continue
                e_t = etp[g % 3]
                nc.scalar.activation(
                    e_t[:], sp[:], mybir.ActivationFunctionType.Exp,
                    bias=0.0, scale=SCALE,
                )
                if stage == "se2":
                    continue
                for s in range(2):
                    blk = (2 * g + s) % (4 * n_kv)
                    nc.tensor.matmul(
                        accp[:],
                        v_sb[:, (D + 1) * blk : (D + 1) * (blk + 1)],
                        e_t[:, CH * s : CH * (s + 1)],
                        start=(done == 0), stop=(done == n_units - 1),
                    )
                    done += 1
            return
        if stage in ("sco", "se", "sea"):
            # scores only / +exp / +AV, 36 pair-groups, no masks/epilogue
            acc = ps_acc.tile([D + 1, CH], FP32, tag="acc", name="acc")
            n_units = 72
            done = 0
            for g in range(36):
                sp = ps_sp.tile([128, 2 * CH], FP32, tag="sp", name="sp")
                for s in range(2):
                    half = 64 * s
                    blk = (2 * g + s) % (4 * n_kv)
                    nc.tensor.matmul(
                        sp[:, CH * s : CH * (s + 1)],
                        kv_sb[half : half + 64, NB * blk : NB * (blk + 1)],
                        qtd[half : half + 64, 0:CH],
                        start=True, stop=True,
                    )
                if stage == "sco":
                    continue
                e_t = exp_pool.tile([128, 2 * CH], BF16, tag="et", name="e_t")
                nc.scalar.activation(
                    e_t[:], sp[:], mybir.ActivationFunctionType.Exp,
                    bias=0.0, scale=SCALE,
                )
                if stage == "se":
                    continue
                for s in range(2):
                    blk = (2 * g + s) % (4 * n_kv)
                    nc.tensor.matmul(
                        acc[:],
                        v_sb[:, (D + 1) * blk : (D + 1) * (blk + 1)],
                        e_t[:, CH * s : CH * (s + 1)],
                        start=(done == 0), stop=(done == n_units - 1),
                    )
                    done += 1
            if stage == "sco":
                # keep sp tiles "consumed" so releases are valid
                pass
            return

    MICRO_STAGES = ("empty", "mm", "mmt", "exp", "att", "attmmt", "sco", "se", "sea",
                    "sco2", "se2", "sea2", "scoK", "scoE", "scoC", "scoS",
                    "mm1", "mm2", "mm3", "mm4", "mm5", "mm6")

    def one_rep():
        if stage in MICRO_STAGES:
            return one_rep_micro()
        # input chunks arrive in dma_group-sized grouped DMAs issued upfront
        # (the ~2us fixed cost per dma_start does not overlap on a ring, so
        # fewer/bigger transfers win); nothing ever blocks the input stream.
        xcs = []
        rings = [nc.sync, nc.scalar, nc.gpsimd][: max(1, dma_rings)]
        if not dma_split:
            rings = [nc.sync]
        for gi, g0 in enumerate(range(0, n_kv, dma_group)):
            g = min(dma_group, n_kv - g0)
            xt_grp = xc_pool.tile([128, g * 8 * CH], BF16, tag="xc", name="xc")
            src = xT[128 * g0 : 128 * (g0 + g), :].rearrange("(g p) m -> p g m", p=128)
            rings[gi % len(rings)].dma_start(
                xt_grp[:].rearrange("p (g m) -> p g m", g=g), src
            )
            for i in range(g):
                xcs.append(xt_grp[:, 8 * CH * i : 8 * CH * (i + 1)])
        if stage == "dma":
            return
        osb_all = sm_pool.tile([D + 1, 4 * CH], FP32, tag="osb", name="osb_all")

        from collections import deque

        gens = deque()

        def pump(n):
            for _ in range(n):
                if not gens:
                    return
                try:
                    next(gens[0])
                except StopIteration:
                    gens.popleft()

        for c in range(n_kv):
            xchunk = xcs[c]
            wstack = wa_sb if c % 2 == 0 else wb_sb
            vhalf = 64 * (1 - (c % 2))  # partition base of vT in kv psum

            kv_ps = ps_pr.tile([128, CH], FP32, tag="prj", name="kv_ps")
            for eb in range(8):
                nc.tensor.matmul(
                    kv_ps[:],
                    wstack[:, 128 * eb : 128 * (eb + 1)],
                    xchunk[:, CH * eb : CH * (eb + 1)],
                    start=(eb == 0),
                    stop=(eb == 7),
                )
            # folded kT/vT for this chunk -> persistent kv_sb (single copy)
            nc.vector.tensor_copy(kv_sb[:, CH * c : CH * (c + 1)], kv_ps[:])
            pump(pump_n)

            # v blocks: PE-transpose the four [64,128] vT blocks into ONE
            # single-bank bf16 psum tile (shares banks with the acc pool),
            # evacuated by a single strided DVE copy.
            vt4 = ps_vt.tile([128, 4 * D], BF16, tag=vt_tag, name="vt4")
            for j in range(4):
                nc.tensor.transpose(
                    vt4[:, D * j : D * (j + 1)],
                    kv_sb[vhalf : vhalf + 64, CH * c + NB * j : CH * c + NB * (j + 1)],
                    ident[vhalf : vhalf + 64, vhalf : vhalf + 64],
                )
            v_dst = v_sb[:].rearrange("p (b c) -> p b c", c=D + 1)[
                :, 4 * c : 4 * (c + 1), 0:D
            ]
            nc.vector.tensor_copy(v_dst, vt4[:].rearrange("p (b c) -> p b c", c=D))
            pump(pump_n)

            if c in q_local:
                qi = q_local[c]
                q_ps = ps_pr.tile([128, CH], FP32, tag="prj", name="q_ps")
                for eb in range(8):
                    nc.tensor.matmul(
                        q_ps[:],
                        wq_sb[:, 128 * eb : 128 * (eb + 1)],
                        xchunk[:, CH * eb : CH * (eb + 1)],
                        start=(eb == 0),
                        stop=(eb == 7),
                    )
                nc.vector.tensor_copy(qtd[:, CH * qi : CH * (qi + 1)], q_ps[:])
                if stage not in ("proj",):
                    gens.append(attention_gen(c, osb_all))
            pump(pump_n)
        while gens:
            pump(100)
        if stage not in ("proj",) and out_combined:
            nc.scalar.dma_start(
                out[:].rearrange("(q p) m -> p q m", p=D + 1),
                osb_all[:].rearrange("p (q m) -> p q m", q=len(q_gids)),
            )

    if hw_loop and repeat > 1:
        u = unroll if repeat % unroll == 0 else 1
        with tc.For_i(0, repeat // u, 1):
            for _ in range(u):
                one_rep()
    else:
        for _rep in range(repeat):
            one_rep()


def build_program(n_kv, q_gids, num_devices=4, repeat=1, hw_loop=False, stage="full",
                  **knobs):
    import contextlib

    nc = bacc.Bacc(
        "TRN2", target_bir_lowering=False, debug=False, num_devices=num_devices
    )
    xT = nc.dram_tensor("xT", [n_kv * 128, 8 * CH], BF16, kind="ExternalInput").ap()
    wa = nc.dram_tensor("wa", [128, E], BF16, kind="ExternalInput").ap()
    wb = nc.dram_tensor("wb", [128, E], BF16, kind="ExternalInput").ap()
    wq2 = nc.dram_tensor("wq2", [128, E], BF16, kind="ExternalInput").ap()
    out = nc.dram_tensor(
        "out", [len(q_gids) * (D + 1), CH], FP32, kind="ExternalOutput"
    ).ap()
    with tile.TileContext(nc) as tc:
        with contextlib.ExitStack() as ctx:
            _build_body(ctx, tc, xT, wa, wb, wq2, out, n_kv, q_gids,
                        repeat=repeat, hw_loop=hw_loop, stage=stage, **knobs)
    nc.compile()
    return nc


# ---------------- host-side runner ----------------


def _make_runner(nc, devices, donate=True):
    import jax
    from jax.experimental.shard_map import shard_map
    from jax.sharding import Mesh, PartitionSpec

    from concourse import bass2jax

    bass2jax.install_neuronx_cc_hook()

    fn0 = nc.m.functions[0]
    partition_name = nc.partition_id_tensor.name if nc.partition_id_tensor else None
    in_names, out_names, out_avals = [], [], []
    for alloc in fn0.allocations:
        if not isinstance(alloc, mybir.MemoryLocationSet):
            continue
        if alloc.kind not in ("ExternalInput", "ExternalOutput"):
            continue
        name = alloc.memorylocations[0].name
        if alloc.kind == "ExternalInput":
            if name != partition_name:
                in_names.append(name)
        else:
            out_names.append(name)
            out_avals.append(
                jax.core.ShapedArray(
                    tuple(alloc.tensor_shape), mybir.dt.np(alloc.dtype)
                )
            )
    n_params = len(in_names)
    n_outs = len(out_names)
    all_names = list(in_names) + list(out_names)
    if partition_name is not None:
        all_names.append(partition_name)
    all_names = tuple(all_names)

    def _body(*args):
        operands = list(args)
        if partition_name is not None:
            operands.append(bass2jax.partition_id_tensor())
        outs = bass2jax._bass_exec_p.bind(
            *operands,
            out_avals=tuple(out_avals),
            in_names=all_names,
            out_names=tuple(out_names),
            lowering_input_output_aliases=(),
            sim_require_finite=True,
            sim_require_nnan=True,
            nc=nc,
        )
        return tuple(outs)

    n_cores = len(devices)
    mesh = Mesh(np.asarray(devices), ("core",))
    in_specs = (PartitionSpec("core"),) * (n_params + n_outs)
    out_specs = (PartitionSpec("core"),) * n_outs
    donate_idx = tuple(range(n_params, n_params + n_outs)) if donate else ()
    sharded = jax.jit(
        shard_map(
            _body, mesh=mesh, in_specs=in_specs, out_specs=out_specs, check_rep=False
        ),
        donate_argnums=donate_idx,
        keep_unused=True,
    )
    return {
        "fn": sharded,
        "in_names": in_names,
        "out_names": out_names,
        "out_avals": out_avals,
        "n_cores": n_cores,
        "nc": nc,
        "devices": devices,
    }


@functools.lru_cache(maxsize=1)
def _get_programs():
    import jax

    devs = jax.devices()
    assert len(devs) >= 8, f"need 8 neuron cores, have {devs}"
    nc_outer = build_program(OUTER_NKV, OUTER_GIDS)
    nc_middle = build_program(MIDDLE_NKV, MIDDLE_GIDS)
    run_outer = _make_runner(nc_outer, devs[0:4])
    run_middle = _make_runner(nc_middle, devs[4:8])
    return run_outer, run_middle


def _concat_inputs(runner, per_core_maps):
    arrs = []
    for name in runner["in_names"]:
        arrs.append(np.concatenate([m[name] for m in per_core_maps], axis=0))
    for av in runner["out_avals"]:
        arrs.append(np.zeros((runner["n_cores"] * av.shape[0], *av.shape[1:]), av.dtype))
    return arrs


def _split_outputs(runner, out_arrs):
    res = []
    for c in range(runner["n_cores"]):
        m = {}
        for i, name in enumerate(runner["out_names"]):
            shp = runner["out_avals"][i].shape
            m[name] = np.asarray(out_arrs[i]).reshape(
                runner["n_cores"], *shp
            )[c]
        res.append(m)
    return res


def _bf16(a):
    import ml_dtypes

    return np.asarray(a, dtype=ml_dtypes.bfloat16)


def pack_x(xb, n_kv):
    """Pack x rows [0:512*n_kv) of one batch into the chunk-major DMA layout:
    out[c*128 + p, eb*512 + t] = xb[512*c + t, 128*eb + p]  (bf16)."""
    arr = xb[: CH * n_kv].reshape(n_kv, CH, 8, 128)
    return np.ascontiguousarray(
        _bf16(arr.transpose(0, 3, 2, 1).reshape(n_kv * 128, 8 * CH))
    )


def stack_w(w1, w2):
    """[128, 8*128] bf16: cols [128*eb : 128*eb+64] = w1 block eb, rest w2."""
    a = w1.reshape(8, 128, D).transpose(1, 0, 2)  # [128, 8, 64]
    b = w2.reshape(8, 128, D).transpose(1, 0, 2)
    return np.ascontiguousarray(
        _bf16(np.concatenate([a, b], axis=2).reshape(128, 8 * 128))
    )


def make_core_inputs(x, Wq, Wk, Wv):
    x = np.asarray(x, dtype=np.float32)
    Wq = np.asarray(Wq, dtype=np.float32)
    Wk = np.asarray(Wk, dtype=np.float32)
    Wv = np.asarray(Wv, dtype=np.float32)
    wa = stack_w(Wk, Wv)
    wb = stack_w(Wv, Wk)
    wq2 = stack_w(Wq, Wq)
    outer_maps, middle_maps = [], []
    for b in range(B):
        outer_maps.append(
            {"xT": pack_x(x[b], OUTER_NKV), "wa": wa, "wb": wb, "wq2": wq2}
        )
        middle_maps.append(
            {"xT": pack_x(x[b], MIDDLE_NKV), "wa": wa, "wb": wb, "wq2": wq2}
        )
    return outer_maps, middle_maps


def assemble_output(outer_res, middle_res):
    out = np.empty((B, T, D), dtype=np.float32)
    for b in range(B):
        for res, gids in ((outer_res, OUTER_GIDS), (middle_res, MIDDLE_GIDS)):
            oc = res[b]["out"]  # [4*65, 512] = unnormalized outT per q-chunk
            for qi, g in enumerate(gids):
                blk = oc[(D + 1) * qi : (D + 1) * (qi + 1)]
                out[b, CH * g : CH * (g + 1)] = (blk[0:D] / blk[D : D + 1]).T
    return out


def kernel(x, Wq, Wk, Wv):
    run_outer, run_middle = _get_programs()
    outer_maps, middle_maps = make_core_inputs(x, Wq, Wk, Wv)
    a_in = _concat_inputs(run_outer, outer_maps)
    b_in = _concat_inputs(run_middle, middle_maps)
    a_out = run_outer["fn"](*a_in)  # async dispatch
    b_out = run_middle["fn"](*b_in)
    outer_res = _split_outputs(run_outer, a_out)
    middle_res = _split_outputs(run_middle, b_out)
    return assemble_output(outer_res, middle_res)


if __name__ == "__main__":
    rng = np.random.default_rng(0)
    x = rng.standard_normal((B, T, E), dtype=np.float32)
    s = 1.0 / np.sqrt(E)
    Wq = rng.uniform(-s, s, (E, D)).astype(np.float32)
    Wk = rng.uniform(-s, s, (E, D)).astype(np.float32)
    Wv = rng.uniform(-s, s, (E, D)).astype(np.float32)
    out = kernel(x, Wq, Wk, Wv)
    print("out", out.shape, out.dtype, np.abs(out).mean())
